# revision 56
# baseline (speedup 1.0000x reference)
"""Trainium2 Bass kernel for nn_EncoderDecoderAttention (B=8, N=1024, D=1024, E=128, H=16).

Math (per batch b):
  Q = x @ wq[h]          [N, E]
  K = enc @ wk[h]        [N, E]
  V = enc @ wv[h]        [N, E]
  s = (Q K^T + mask) / sqrt(E)   with mask rows n >= NV set to -inf, NV = min(current_index+1, N-1)
  attn = softmax over the QUERY axis (per key column)
  heads = attn @ V; out = concat_heads @ w_agg

Because masked query rows are -inf before the softmax, attn rows n >= NV are exactly
zero, so output rows n >= NV are exactly zero: the device only computes rows [0, NV).

Sharding: pure data-parallel over batch across the 8 NeuronCores (one batch element
per core, full heads per core, no collectives).

Device layout (per core), NV = 513 fast path:
  The device computes queries 0..511 (every matmul FD=512-aligned). Query 512 only
  feeds (a) the softmax denominators and (b) output row 512; its unnormalized score
  row exp512[h, m] = exp(q512 . K_h[m] / sqrt(E)) is precomputed on the host
  (~0.3 GFLOP of glue) and shipped as a tiny input, so the ragged FD=1 matmuls for
  Q/scores disappear. Per (h, key-tile):
    scoresT = K^T-tile stationary x Q^T  -> psum [128, 512] (one bank)
    exp on scalar engine (fused free-axis accum) -> a_sb bf16 + ssum
    ssum += exp512 column; rcp = 1/ssum (vector); vsc = V-block * rcp
    headsT += vsc x a_sb  (+ FD=1 tail column from exp512 into a shared psum bank)
  The final w_agg matmul is split: heads 0-11 chunks are interleaved into the
  attend drain as PE filler, heads 12-15 finish after the last attend, adding the
  stashed partial on the vector engine; output streams out bf16 (host upcasts).
"""

import sys

if "/opt/trn_rl_repo" not in sys.path:
    sys.path.insert(0, "/opt/trn_rl_repo")

import ml_dtypes
import numpy as np

import concourse.mybir as mybir
import concourse.tile as tile
from concourse import bacc
from concourse.bass_utils import run_bass_kernel_spmd

B, N, D, E, H = 8, 1024, 1024, 128, 16
P = 128
KD = D // P  # contraction tiles over D
MT = N // P  # key tiles over N
NCORES = 8
BF16 = mybir.dt.bfloat16
FP32 = mybir.dt.float32

# test.py can flip these to profile
TRACE = False
LAST_RESULTS = None

_cache = {}


def _ensure_ntff_hook():
    """Register the axon NTFF profiling hook if the boot shim couldn't.

    Adapted from trn_agent_boot/trn_boot.py: the agent image's ``antenv``
    package lacks ``axon_hooks``, so ``trace=True`` silently skips NTFF
    capture. Inject an equivalent module backed by ctypes calls into the
    axon PJRT .so. Also neuter ``upload_artifacts`` (zero-egress box).
    """
    import contextlib
    import ctypes
    import os
    import types

    try:
        from antenv.axon_hooks import get_axon_ntff_profile_hook  # noqa: F401

        return
    except ImportError:
        pass

    so_path = "/opt/axon/libaxon_pjrt.so"
    if not os.path.exists(so_path):
        return
    lib = ctypes.CDLL(so_path)
    if not hasattr(lib, "axon_start_nrt_profile"):
        return
    lib.axon_start_nrt_profile.argtypes = [
        ctypes.POINTER(ctypes.c_int64),
        ctypes.c_size_t,
    ]
    lib.axon_start_nrt_profile.restype = ctypes.c_int64
    lib.axon_stop_nrt_profile.argtypes = [ctypes.c_char_p]
    lib.axon_stop_nrt_profile.restype = ctypes.c_int64

    @contextlib.contextmanager
    def _hook(output_dir, device_ids):
        import jax

        jax.devices()
        if device_ids:
            ids = (ctypes.c_int64 * len(device_ids))(*device_ids)
            rc = lib.axon_start_nrt_profile(ids, len(device_ids))
        else:
            rc = lib.axon_start_nrt_profile(None, 0)
        if rc != 0:
            raise RuntimeError(f"axon_start_nrt_profile rc={rc}")
        try:
            yield
        finally:
            n = lib.axon_stop_nrt_profile(str(output_dir).encode())
            print(f"ntff profile: {n} file(s) -> {output_dir}", file=sys.stderr)

    mod = types.ModuleType("antenv.axon_hooks")
    mod.get_axon_ntff_profile_hook = lambda: _hook
    mod.set_axon_ntff_profile_hook = lambda h: None
    sys.modules["antenv.axon_hooks"] = mod

    # upload_artifacts reaches for a bucket; keep everything local.
    from concourse import bass_utils as _bu

    _orig_upload = _bu.upload_artifacts

    def _safe_upload(tmpdir):
        try:
            return _orig_upload(tmpdir)
        except Exception:
            return str(tmpdir)

    _bu.upload_artifacts = _safe_upload


def _chunks(total, step):
    return [(s, min(step, total - s)) for s in range(0, total, step)]


def _build(NV):
    """Fast path for NV = k*128 + 1 (the shipped case: NV=513)."""
    NDEV = NV - 1  # device-computed query rows, tile-aligned
    nc = bacc.Bacc("TRN2", target_bir_lowering=False, debug=False, num_devices=NCORES)

    xT_d = nc.dram_tensor("xT", [P, KD, NDEV], BF16, kind="ExternalInput")
    encT_d = nc.dram_tensor("encT", [P, KD, N], BF16, kind="ExternalInput")
    wq_d = nc.dram_tensor("wq", [P, H, KD, E], BF16, kind="ExternalInput")
    wk_d = nc.dram_tensor("wk", [P, H, KD, E], BF16, kind="ExternalInput")
    wv_d = nc.dram_tensor("wv", [P, 4, KD, H // 4, E], BF16, kind="ExternalInput")
    wagg_d = nc.dram_tensor("wagg", [P, H, D], BF16, kind="ExternalInput")
    # exp of the tail query's score row, keys on partitions: [m%P, mt, h]
    e512_d = nc.dram_tensor("e512", [P, MT, H], BF16, kind="ExternalInput")
    out_d = nc.dram_tensor("out", [NDEV, D], BF16, kind="ExternalOutput")
    tail_d = nc.dram_tensor("tail_he", [P, H], BF16, kind="ExternalOutput")

    n_tiles = _chunks(NDEV, P)
    he_chunks = _chunks(H * E, 512)
    d_chunks = _chunks(D, 512)
    m_chunks = _chunks(N, 512)
    scale = 1.0 / float(np.sqrt(E))

    DEPTH = 2
    H1 = 8  # final w_agg split: 0..H1-1 / H1..13 / 14-15 (see emit_final_chunk)

    with tile.TileContext(nc) as tc:
        with (
            tc.tile_pool(name="persist", bufs=1) as persist,
            tc.tile_pool(name="wgroup", bufs=2) as wgroup,
            tc.tile_pool(name="work", bufs=3) as work,
            tc.tile_pool(name="apool", bufs=5) as apool,
            tc.tile_pool(name="stats", bufs=8) as stats,
            tc.tile_pool(name="opool", bufs=3) as opool,
            tc.tile_pool(name="fpool", bufs=1) as fpool,
            tc.tile_pool(name="ps2", bufs=2, space="PSUM") as ps2,
            tc.tile_pool(name="psnv", bufs=3, space="PSUM") as psnv,
            tc.tile_pool(name="psq", bufs=1, space="PSUM") as psq,
            tc.tile_pool(name="psacc", bufs=1, space="PSUM") as psacc,
            tc.tile_pool(name="pstail", bufs=1, space="PSUM") as pstail,
        ):
            # Warm the PE clock gate ASAP with a short dependency-light dummy
            # burst (the PE queue is in-order, so a long burst would delay the
            # first real matmuls instead).
            scratch = persist.tile([P, 256], BF16, name="warm_scratch")
            nc.vector.memset(scratch[:], 0.0)
            dpsA = ps2.tile([P, 512], FP32, tag="ps512", name="dpsA")
            dpsB = ps2.tile([P, 512], FP32, tag="ps512", name="dpsB")
            for i in range(8):
                nc.tensor.matmul(
                    (dpsA if i % 2 == 0 else dpsB)[:, :256],
                    scratch[:, :P],
                    scratch[:],
                    start=True,
                    stop=True,
                    skip_group_check=True,
                )

            # DMA issues serialize at ~700ns each on the queue, so use FEW,
            # LARGE transfers (host pre-lays everything out contiguously),
            # ordered by on-device deadline.
            xT = persist.tile([P, KD, NDEV], BF16, name="xT_sb")
            encT = persist.tile([P, KD, N], BF16, name="encT_sb")
            e512 = persist.tile([P, MT, H], BF16, name="e512_sb")
            wv = persist.tile([P, 4, KD, H // 4, E], BF16, name="wv_sb")
            wagg = persist.tile([P, H, D], BF16, name="wagg_sb")
            # wq/wk stream through rotating 4-head group tiles (one DMA issue
            # per group; a group's issue stalls the queue until its buffer
            # frees, so late groups are ordered after everything early)
            wqg = [wgroup.tile([P, 4, KD, E], BF16, tag="wq", name=f"wqg{g}")
                   for g in range(4)]
            wkg = [wgroup.tile([P, 4, KD, E], BF16, tag="wk", name=f"wkg{g}")
                   for g in range(4)]

            nc.sync.dma_start(wqg[0][:, 0:1], wq_d[:, 0:1])
            nc.sync.dma_start(xT[:, 0:2, :], xT_d[:, 0:2, :])
            nc.sync.dma_start(xT[:, 2:4, :], xT_d[:, 2:4, :])
            nc.sync.dma_start(xT[:, 4:6, :], xT_d[:, 4:6, :])
            nc.sync.dma_start(xT[:, 6:KD, :], xT_d[:, 6:KD, :])
            nc.sync.dma_start(wkg[0][:, 0:1], wk_d[:, 0:1])
            nc.sync.dma_start(encT[:, 0:2, :], encT_d[:, 0:2, :])
            nc.sync.dma_start(encT[:, 2:4, :], encT_d[:, 2:4, :])
            nc.sync.dma_start(encT[:, 4:6, :], encT_d[:, 4:6, :])
            nc.sync.dma_start(encT[:, 6:KD, :], encT_d[:, 6:KD, :])
            nc.sync.dma_start(wqg[0][:, 1:4], wq_d[:, 1:4])
            nc.sync.dma_start(wkg[0][:, 1:4], wk_d[:, 1:4])
            nc.sync.dma_start(e512[:], e512_d[:])
            nc.sync.dma_start(wqg[1][:], wq_d[:, 4:8])
            nc.sync.dma_start(wkg[1][:], wk_d[:, 4:8])
            nc.sync.dma_start(wv[:, 0], wv_d[:, 0])
            nc.sync.dma_start(wv[:, 1], wv_d[:, 1])
            nc.sync.dma_start(wv[:, 2], wv_d[:, 2])
            nc.sync.dma_start(wv[:, 3], wv_d[:, 3])
            nc.sync.dma_start(wagg[:], wagg_d[:])
            nc.sync.dma_start(wqg[2][:], wq_d[:, 8:12])
            nc.sync.dma_start(wkg[2][:], wk_d[:, 8:12])
            nc.sync.dma_start(wqg[3][:], wq_d[:, 12:16])
            nc.sync.dma_start(wkg[3][:], wk_d[:, 12:16])

            vall = persist.tile([P, MT, H * E], BF16, name="vall_sb")
            multiT = persist.tile([P, H, NDEV], BF16, name="multiT_sb")
            htail = pstail.tile([P, H], FP32, name="htail_ps")

            qts = {}
            kts = {}

            def make_proj(h):
                """Per-matmul emitters for head h's Q/K projections, to be
                interleaved into an attend's iterations so the PE's OOO window
                always holds chain-independent work. Returns (q_mms, k_mms,
                proj_fin); proj_fin emits the qt cast. The K psum tile is
                allocated lazily per m-chunk (m-chunk-outer, kd-inner) and
                cast as soon as its chunk finishes, so only one ps2 buffer is
                held at a time -- the other rotates with deferred-V tiles."""
                qt = work.tile([P, NDEV], BF16, tag="qt", name="qt")
                qps = psq.tile([P, NDEV], FP32, tag="psq", name="qps")
                kt = work.tile([P, N], BF16, tag="kt", name="kt")
                kstate = {}

                def q_mm(kd):
                    nc.tensor.matmul(
                        qps[:],
                        wqg[h // 4][:, h % 4, kd, :],
                        xT[:, kd, :],
                        start=(kd == 0),
                        stop=(kd == KD - 1),
                        skip_group_check=True,
                    )

                def k_mm(j):
                    i, kd = j // KD, j % KD
                    ms, ml = m_chunks[i]
                    if kd == 0:
                        kstate[i] = ps2.tile([P, 512], FP32, tag="ps512",
                                             name=f"kps{i}")
                    nc.tensor.matmul(
                        kstate[i][:, :ml],
                        wkg[h // 4][:, h % 4, kd, :],
                        encT[:, kd, ms : ms + ml],
                        start=(kd == 0),
                        stop=(kd == KD - 1),
                        skip_group_check=True,
                    )
                    if kd == KD - 1:
                        nc.vector.tensor_copy(
                            out=kt[:, ms : ms + ml], in_=kstate.pop(i)[:, :ml]
                        )

                def proj_fin():
                    nc.vector.tensor_copy(out=qt[:], in_=qps[:])
                    qts[h] = qt
                    kts[h] = kt

                q_mms = [(lambda kd: lambda: q_mm(kd))(kd) for kd in range(KD)]
                k_mms = [(lambda j: lambda: k_mm(j))(j) for j in range(2 * KD)]
                return q_mms, k_mms, proj_fin

            def emit_proj(h):
                # standalone proj for the first DEPTH heads (the rest are
                # interleaved into attend iterations)
                q_mms, k_mms, proj_fin = make_proj(h)
                for f in q_mms:
                    f()
                for f in k_mms:
                    f()
                proj_fin()

            def make_v_quarter(q):
                """Per-matmul emitters for the V projection of he-quarter q
                (heads 4q..4q+3): vall[m%P, mt, 512q:512(q+1)]. The psum tile
                allocates lazily per key-tile and casts on its last matmul,
                holding one ps2 buffer at a time."""
                state = {}

                def v_mm(j):
                    mt, kd = j // KD, j % KD
                    if kd == 0:
                        state[mt] = ps2.tile([P, 512], FP32, tag="ps512",
                                             name="vps")
                    nc.tensor.matmul(
                        state[mt][:],
                        encT[:, kd, mt * P : (mt + 1) * P],
                        wv[:, q, kd],
                        start=(kd == 0),
                        stop=(kd == KD - 1),
                        skip_group_check=True,
                    )
                    if kd == KD - 1:
                        nc.vector.tensor_copy(
                            out=vall[:, mt, q * 512 : (q + 1) * 512],
                            in_=state.pop(mt)[:],
                        )

                return [(lambda j: lambda: v_mm(j))(j) for j in range(MT * KD)]

            def emit_v_phase(quarters):
                for q in quarters:
                    for f in make_v_quarter(q):
                        f()

            def emit_attend(h, q_mms=(), k_mms=(), v_mms=(), fillers=()):
                # scores^T, softmax over free axis, headsT accum over key
                # tiles. The heads matmul is emitted DELAY iterations behind
                # the scores matmul (its stationary vsc comes off the softmax
                # chain ~2us later), and the next head's Q/K proj matmuls are
                # interleaved per-iteration so the PE's OOO exec window always
                # holds chain-independent work.
                DELAY = 3
                q_mms = list(q_mms)
                k_mms = list(k_mms)
                v_mms = list(v_mms)
                fillers = list(fillers)
                qt = qts.pop(h)
                kt = kts.pop(h)
                hps = psacc.tile([P, NDEV], FP32, tag="hacc", name="hps")
                abuf = {}
                vbuf = {}

                def emit_heads(mt):
                    nc.tensor.matmul(
                        hps[:],
                        vbuf[mt][:],
                        abuf.pop(mt)[:],
                        start=(mt == 0),
                        stop=(mt == MT - 1),
                        skip_group_check=True,
                    )
                    # tail output row: heads[512] column accumulates in a
                    # shared psum bank (read once after the last attend)
                    nc.tensor.matmul(
                        htail[:, h : h + 1],
                        vbuf.pop(mt)[:],
                        e512[:, mt, h : h + 1],
                        start=(mt == 0),
                        stop=(mt == MT - 1),
                        skip_group_check=True,
                    )

                for mt in range(MT):
                    tps = psnv.tile([P, NDEV], FP32, tag="psnv", name="tps")
                    nc.tensor.matmul(
                        tps[:],
                        kt[:, mt * P : (mt + 1) * P],
                        qt[:],
                        start=True,
                        stop=True,
                    )
                    a_sb = apool.tile([P, NDEV], BF16, tag="a", name="a_sb")
                    ssum = stats.tile([P, 1], FP32, tag="ssum", name="ssum")
                    nc.scalar.activation(
                        a_sb[:],
                        tps[:],
                        mybir.ActivationFunctionType.Exp,
                        scale=scale,
                        accum_out=ssum[:],
                    )
                    # denominators include the host tail-query column
                    ssumt = stats.tile([P, 1], FP32, tag="ssumt", name="ssumt")
                    nc.vector.tensor_tensor(
                        ssumt[:], ssum[:], e512[:, mt, h : h + 1],
                        mybir.AluOpType.add,
                    )
                    rcp = stats.tile([P, 1], FP32, tag="rcp", name="rcp")
                    nc.vector.reciprocal(rcp[:], ssumt[:])
                    vsc = apool.tile([P, E], BF16, tag="vsc", name="vsc")
                    nc.vector.tensor_scalar_mul(
                        vsc[:], vall[:, mt, h * E : (h + 1) * E], rcp[:]
                    )
                    abuf[mt] = a_sb
                    vbuf[mt] = vsc
                    if q_mms:
                        q_mms.pop(0)()
                    if mt >= DELAY:
                        emit_heads(mt - DELAY)
                    if k_mms:
                        k_mms.pop(0)()
                        k_mms.pop(0)()
                    if v_mms:
                        v_mms.pop(0)()
                        v_mms.pop(0)()
                    if fillers and mt % 2 == 1:
                        fillers.pop(0)()

                def finish():
                    # last DELAY heads matmuls + the multiT copy; the trailing
                    # chain latency hides under the next head's independent
                    # scores/proj matmuls via the PE OOO window.
                    for mt in range(MT - DELAY, MT):
                        emit_heads(mt)
                    nc.vector.tensor_copy(out=multiT[:, h, :], in_=hps[:])
                    for f in fillers:
                        f()

                return finish

            fin_parts = {}

            def emit_final_chunk(ns, nl, ds_, dl, part):
                # out[n, d] = concat_heads @ w_agg, 3-way head split: part 0
                # (heads 0..H1-1) runs as attends-8..13 filler and stashes a
                # bf16 partial; part 1 (heads H1..13) runs as drain filler and
                # merges in-place on the vector engine; part 2 (heads 14-15)
                # finishes after the last attend -- just 2 matmuls + add + DMA
                # per chunk on the tail.
                fps = ps2.tile([P, 512], FP32, tag="ps512", name="fps")
                hts = [range(0, H1), range(H1, H - 2), range(H - 2, H)][part]
                for ht in hts:
                    nc.tensor.matmul(
                        fps[:nl, :dl],
                        multiT[:, ht, ns : ns + nl],
                        wagg[:, ht, ds_ : ds_ + dl],
                        start=(ht == hts[0]),
                        stop=(ht == hts[-1]),
                    )
                key = (ns // P) * 2 + ds_ // 512
                if part == 0:
                    pa = fpool.tile([P, 512], BF16, tag=f"part{key}", name="pa")
                    nc.vector.tensor_copy(out=pa[:nl, :dl], in_=fps[:nl, :dl])
                    fin_parts[key] = pa
                elif part == 1:
                    pa = fin_parts[key]
                    nc.vector.tensor_tensor(
                        pa[:nl, :dl], fps[:nl, :dl], pa[:nl, :dl],
                        mybir.AluOpType.add,
                    )
                else:
                    osb = opool.tile([P, 512], BF16, tag="osb", name="osb")
                    pa = fin_parts.pop(key)
                    nc.vector.tensor_tensor(
                        osb[:nl, :dl],
                        fps[:nl, :dl],
                        pa[:nl, :dl],
                        mybir.AluOpType.add,
                    )
                    nc.sync.dma_start(out_d[ns : ns + nl, ds_ : ds_ + dl], osb[:nl, :dl])

            # Software pipeline: proj(h) runs DEPTH ahead of attend(h); the V
            # phase covers the encT/wv DMA stream. Each attend interleaves the
            # (h+DEPTH) head's proj matmuls per-iteration; trailing heads
            # matmuls are deferred into the next attend's start.
            for h in range(DEPTH):
                emit_proj(h)
            # V quarters 0-1 upfront (attends 0-7 read them); quarters 2-3
            # are deferred into attends 0-7 as an extra interleave stream so
            # the PE isn't gated on the tail of the wv DMA
            emit_v_phase([0, 1])
            vdef = make_v_quarter(2) + make_v_quarter(3)
            all_chunks = [
                (ns, nl, ds_, dl) for ns, nl in n_tiles for ds_, dl in d_chunks
            ]
            # final part-0 chunks (heads 0..H1-1, ready after attend 7) feed
            # the attends that have no deferred-V interleave left
            p0 = [(lambda c: lambda: emit_final_chunk(*c, 0))(c)
                  for c in all_chunks]
            p0_share = {8: 2, 9: 2, 10: 1, 11: 1, 12: 1, 13: 1}
            for h in range(DEPTH, H):
                a = h - DEPTH
                q_mms, k_mms, proj_fin = make_proj(h)
                share, vdef = vdef[: 2 * MT], vdef[2 * MT :]
                fils, p0 = p0[: p0_share.get(a, 0)], p0[p0_share.get(a, 0) :]
                fin_new = emit_attend(h - DEPTH, q_mms=q_mms, k_mms=k_mms,
                                      v_mms=share, fillers=fils)
                proj_fin()
                fin_new()
            # Drain: the last DEPTH attends have no proj work left; final
            # part-1 chunks (heads H1..13, multiT ready after attend 13) fill.
            drain = list(range(H - DEPTH, H))
            per = (len(all_chunks) + len(drain) - 1) // len(drain)
            for i, h in enumerate(drain):
                cs = all_chunks[i * per : (i + 1) * per]
                fils = [(lambda c: lambda: emit_final_chunk(*c, 1))(c) for c in cs]
                fin = emit_attend(h, fillers=fils[:-1])
                fils[-1]()
                fin()
            # ship the tail heads column while the last final chunks run
            tailc = opool.tile([P, H], BF16, tag="tailc", name="tailc")
            nc.vector.tensor_copy(out=tailc[:], in_=htail[:])
            nc.gpsimd.dma_start(tail_d[:], tailc[:])
            for c in all_chunks:
                emit_final_chunk(*c, 2)

    nc.compile()
    return nc


def kernel(x, encoder_context, attention_mask, wq, wk, wv, w_agg, current_index):
    global LAST_RESULTS
    x = np.asarray(x)
    enc = np.asarray(encoder_context)
    wq = np.asarray(wq)
    wk = np.asarray(wk)
    wv = np.asarray(wv)
    w_agg = np.asarray(w_agg)
    ci = int(np.asarray(current_index))
    NV = min(ci + 1, N - 1)
    NDEV = NV - 1
    assert NV % P == 1 and NV > P, "kernel tuned for NV = k*128 + 1 (spec: 513)"

    nc = _cache.get(NV)
    if nc is None:
        nc = _build(NV)
        _cache[NV] = nc

    bf = ml_dtypes.bfloat16
    # weight layouts: see dram tensor declarations in _build
    wq_h = np.ascontiguousarray(wq.reshape(H, KD, P, E).transpose(2, 0, 1, 3)).astype(bf)
    wk_h = np.ascontiguousarray(wk.reshape(H, KD, P, E).transpose(2, 0, 1, 3)).astype(bf)
    wv_h = np.ascontiguousarray(
        wv.reshape(4, H // 4, KD, P, E).transpose(3, 0, 2, 1, 4)
    ).astype(bf)
    wagg_h = np.ascontiguousarray(w_agg.reshape(H, P, D).transpose(1, 0, 2)).astype(bf)

    scale = 1.0 / np.sqrt(np.float32(E))
    in_maps = []
    for b in range(B):
        xT_b = np.ascontiguousarray(
            x[b, :NDEV, :].T.reshape(KD, P, NDEV).transpose(1, 0, 2)
        ).astype(bf)
        encT_b = np.ascontiguousarray(
            enc[b].T.reshape(KD, P, N).transpose(1, 0, 2)
        ).astype(bf)
        # Tail-query score row, computed exactly on the host:
        #   q512[h] = x[512] @ wq[h];  s512[h, m] = enc[m] . (wk[h] @ q512[h])
        q512 = np.einsum("d,hde->he", x[b, NDEV], wq, optimize=True)
        t = np.einsum("hde,he->hd", wk, q512, optimize=True)
        s512 = enc[b].astype(np.float32) @ t.T.astype(np.float32)  # [M, H]
        e512_b = np.ascontiguousarray(
            np.exp(s512 * scale).reshape(MT, P, H).transpose(1, 0, 2)
        ).astype(bf)
        in_maps.append(
            {
                "xT": xT_b,
                "encT": encT_b,
                "wq": wq_h,
                "wk": wk_h,
                "wv": wv_h,
                "wagg": wagg_h,
                "e512": e512_b,
            }
        )

    if TRACE:
        _ensure_ntff_hook()
    res = run_bass_kernel_spmd(
        nc, in_maps, core_ids=list(range(NCORES)), trace=TRACE
    )
    LAST_RESULTS = res

    out = np.zeros((B, N, D), np.float32)
    wagg_f = w_agg.astype(np.float32)
    for b in range(B):
        r = res.results[b]
        out[b, :NDEV, :] = np.asarray(r["out"]).astype(np.float32)
        # tail_he[p, h] = heads[512, h*E + p]
        t = np.asarray(r["tail_he"]).astype(np.float32)
        out[b, NDEV, :] = t.T.reshape(H * E) @ wagg_f
    return out


# revision 57
# speedup vs baseline: 1.0097x; 1.0097x over previous
"""Trainium2 Bass kernel for nn_EncoderDecoderAttention (B=8, N=1024, D=1024, E=128, H=16).

Math (per batch b):
  Q = x @ wq[h]          [N, E]
  K = enc @ wk[h]        [N, E]
  V = enc @ wv[h]        [N, E]
  s = (Q K^T + mask) / sqrt(E)   with mask rows n >= NV set to -inf, NV = min(current_index+1, N-1)
  attn = softmax over the QUERY axis (per key column)
  heads = attn @ V; out = concat_heads @ w_agg

Because masked query rows are -inf before the softmax, attn rows n >= NV are exactly
zero, so output rows n >= NV are exactly zero: the device only computes rows [0, NV).

Sharding: pure data-parallel over batch across the 8 NeuronCores (one batch element
per core, full heads per core, no collectives).

Device layout (per core), NV = 513 fast path:
  The device computes queries 0..511 (every matmul FD=512-aligned). Query 512 only
  feeds (a) the softmax denominators and (b) output row 512; its unnormalized score
  row exp512[h, m] = exp(q512 . K_h[m] / sqrt(E)) is precomputed on the host
  (~0.3 GFLOP of glue) and shipped as a tiny input, so the ragged FD=1 matmuls for
  Q/scores disappear. Per (h, key-tile):
    scoresT = K^T-tile stationary x Q^T  -> psum [128, 512] (one bank)
    exp on scalar engine (fused free-axis accum) -> a_sb bf16 + ssum
    ssum += exp512 column; rcp = 1/ssum (vector); vsc = V-block * rcp
    headsT += vsc x a_sb  (+ FD=1 tail column from exp512 into a shared psum bank)
  The final w_agg matmul is split: heads 0-11 chunks are interleaved into the
  attend drain as PE filler, heads 12-15 finish after the last attend, adding the
  stashed partial on the vector engine; output streams out bf16 (host upcasts).
"""

import sys

if "/opt/trn_rl_repo" not in sys.path:
    sys.path.insert(0, "/opt/trn_rl_repo")

import ml_dtypes
import numpy as np

import concourse.mybir as mybir
import concourse.tile as tile
from concourse import bacc
from concourse.bass_utils import run_bass_kernel_spmd

B, N, D, E, H = 8, 1024, 1024, 128, 16
P = 128
KD = D // P  # contraction tiles over D
MT = N // P  # key tiles over N
NCORES = 8
BF16 = mybir.dt.bfloat16
FP32 = mybir.dt.float32

# test.py can flip these to profile
TRACE = False
LAST_RESULTS = None

_cache = {}


def _ensure_ntff_hook():
    """Register the axon NTFF profiling hook if the boot shim couldn't.

    Adapted from trn_agent_boot/trn_boot.py: the agent image's ``antenv``
    package lacks ``axon_hooks``, so ``trace=True`` silently skips NTFF
    capture. Inject an equivalent module backed by ctypes calls into the
    axon PJRT .so. Also neuter ``upload_artifacts`` (zero-egress box).
    """
    import contextlib
    import ctypes
    import os
    import types

    try:
        from antenv.axon_hooks import get_axon_ntff_profile_hook  # noqa: F401

        return
    except ImportError:
        pass

    so_path = "/opt/axon/libaxon_pjrt.so"
    if not os.path.exists(so_path):
        return
    lib = ctypes.CDLL(so_path)
    if not hasattr(lib, "axon_start_nrt_profile"):
        return
    lib.axon_start_nrt_profile.argtypes = [
        ctypes.POINTER(ctypes.c_int64),
        ctypes.c_size_t,
    ]
    lib.axon_start_nrt_profile.restype = ctypes.c_int64
    lib.axon_stop_nrt_profile.argtypes = [ctypes.c_char_p]
    lib.axon_stop_nrt_profile.restype = ctypes.c_int64

    @contextlib.contextmanager
    def _hook(output_dir, device_ids):
        import jax

        jax.devices()
        if device_ids:
            ids = (ctypes.c_int64 * len(device_ids))(*device_ids)
            rc = lib.axon_start_nrt_profile(ids, len(device_ids))
        else:
            rc = lib.axon_start_nrt_profile(None, 0)
        if rc != 0:
            raise RuntimeError(f"axon_start_nrt_profile rc={rc}")
        try:
            yield
        finally:
            n = lib.axon_stop_nrt_profile(str(output_dir).encode())
            print(f"ntff profile: {n} file(s) -> {output_dir}", file=sys.stderr)

    mod = types.ModuleType("antenv.axon_hooks")
    mod.get_axon_ntff_profile_hook = lambda: _hook
    mod.set_axon_ntff_profile_hook = lambda h: None
    sys.modules["antenv.axon_hooks"] = mod

    # upload_artifacts reaches for a bucket; keep everything local.
    from concourse import bass_utils as _bu

    _orig_upload = _bu.upload_artifacts

    def _safe_upload(tmpdir):
        try:
            return _orig_upload(tmpdir)
        except Exception:
            return str(tmpdir)

    _bu.upload_artifacts = _safe_upload


def _chunks(total, step):
    return [(s, min(step, total - s)) for s in range(0, total, step)]


def _build(NV):
    """Fast path for NV = k*128 + 1 (the shipped case: NV=513)."""
    NDEV = NV - 1  # device-computed query rows, tile-aligned
    nc = bacc.Bacc("TRN2", target_bir_lowering=False, debug=False, num_devices=NCORES)

    xT_d = nc.dram_tensor("xT", [P, KD, NDEV], BF16, kind="ExternalInput")
    encT_d = nc.dram_tensor("encT", [P, KD, N], BF16, kind="ExternalInput")
    wq_d = nc.dram_tensor("wq", [P, H, KD, E], BF16, kind="ExternalInput")
    wk_d = nc.dram_tensor("wk", [P, H, KD, E], BF16, kind="ExternalInput")
    wv_d = nc.dram_tensor("wv", [P, 4, KD, H // 4, E], BF16, kind="ExternalInput")
    wagg_d = nc.dram_tensor("wagg", [P, H, D], BF16, kind="ExternalInput")
    # exp of the tail query's score row, keys on partitions: [m%P, mt, h]
    e512_d = nc.dram_tensor("e512", [P, MT, H], BF16, kind="ExternalInput")
    out_d = nc.dram_tensor("out", [NDEV, D], BF16, kind="ExternalOutput")
    tail_d = nc.dram_tensor("tail_he", [P, H], BF16, kind="ExternalOutput")

    n_tiles = _chunks(NDEV, P)
    he_chunks = _chunks(H * E, 512)
    d_chunks = _chunks(D, 512)
    m_chunks = _chunks(N, 512)
    scale = 1.0 / float(np.sqrt(E))

    DEPTH = 2
    H1 = 12  # final-phase heads computed as drain filler; H-H1 finish at the end

    with tile.TileContext(nc) as tc:
        with (
            tc.tile_pool(name="persist", bufs=1) as persist,
            tc.tile_pool(name="wgroup", bufs=2) as wgroup,
            tc.tile_pool(name="work", bufs=3) as work,
            tc.tile_pool(name="apool", bufs=5) as apool,
            tc.tile_pool(name="stats", bufs=8) as stats,
            tc.tile_pool(name="opool", bufs=3) as opool,
            tc.tile_pool(name="fpool", bufs=1) as fpool,
            tc.tile_pool(name="ps2", bufs=2, space="PSUM") as ps2,
            tc.tile_pool(name="psnv", bufs=3, space="PSUM") as psnv,
            tc.tile_pool(name="psq", bufs=1, space="PSUM") as psq,
            tc.tile_pool(name="psacc", bufs=1, space="PSUM") as psacc,
            tc.tile_pool(name="pstail", bufs=1, space="PSUM") as pstail,
        ):
            # Warm the PE clock gate ASAP with a short dependency-light dummy
            # burst (the PE queue is in-order, so a long burst would delay the
            # first real matmuls instead).
            scratch = persist.tile([P, 256], BF16, name="warm_scratch")
            nc.vector.memset(scratch[:], 0.0)
            dpsA = ps2.tile([P, 512], FP32, tag="ps512", name="dpsA")
            dpsB = ps2.tile([P, 512], FP32, tag="ps512", name="dpsB")
            for i in range(8):
                nc.tensor.matmul(
                    (dpsA if i % 2 == 0 else dpsB)[:, :256],
                    scratch[:, :P],
                    scratch[:],
                    start=True,
                    stop=True,
                    skip_group_check=True,
                )

            # DMA issues serialize at ~700ns each on the queue, so use FEW,
            # LARGE transfers (host pre-lays everything out contiguously),
            # ordered by on-device deadline.
            xT = persist.tile([P, KD, NDEV], BF16, name="xT_sb")
            encT = persist.tile([P, KD, N], BF16, name="encT_sb")
            e512 = persist.tile([P, MT, H], BF16, name="e512_sb")
            wv = persist.tile([P, 4, KD, H // 4, E], BF16, name="wv_sb")
            wagg = persist.tile([P, H, D], BF16, name="wagg_sb")
            # wq/wk stream through rotating 4-head group tiles (one DMA issue
            # per group; a group's issue stalls the queue until its buffer
            # frees, so late groups are ordered after everything early)
            wqg = [wgroup.tile([P, 4, KD, E], BF16, tag="wq", name=f"wqg{g}")
                   for g in range(4)]
            wkg = [wgroup.tile([P, 4, KD, E], BF16, tag="wk", name=f"wkg{g}")
                   for g in range(4)]

            nc.sync.dma_start(wqg[0][:, 0:1], wq_d[:, 0:1])
            nc.sync.dma_start(xT[:, 0:2, :], xT_d[:, 0:2, :])
            nc.sync.dma_start(xT[:, 2:4, :], xT_d[:, 2:4, :])
            nc.sync.dma_start(xT[:, 4:6, :], xT_d[:, 4:6, :])
            nc.sync.dma_start(xT[:, 6:KD, :], xT_d[:, 6:KD, :])
            nc.sync.dma_start(wkg[0][:, 0:1], wk_d[:, 0:1])
            nc.sync.dma_start(encT[:, 0:2, :], encT_d[:, 0:2, :])
            nc.sync.dma_start(encT[:, 2:4, :], encT_d[:, 2:4, :])
            nc.sync.dma_start(encT[:, 4:6, :], encT_d[:, 4:6, :])
            nc.sync.dma_start(encT[:, 6:KD, :], encT_d[:, 6:KD, :])
            nc.sync.dma_start(wqg[0][:, 1:4], wq_d[:, 1:4])
            nc.sync.dma_start(wkg[0][:, 1:4], wk_d[:, 1:4])
            nc.sync.dma_start(e512[:], e512_d[:])
            nc.sync.dma_start(wqg[1][:], wq_d[:, 4:8])
            nc.sync.dma_start(wkg[1][:], wk_d[:, 4:8])
            nc.sync.dma_start(wv[:, 0], wv_d[:, 0])
            nc.sync.dma_start(wv[:, 1], wv_d[:, 1])
            nc.sync.dma_start(wv[:, 2], wv_d[:, 2])
            nc.sync.dma_start(wv[:, 3], wv_d[:, 3])
            nc.sync.dma_start(wagg[:], wagg_d[:])
            nc.sync.dma_start(wqg[2][:], wq_d[:, 8:12])
            nc.sync.dma_start(wkg[2][:], wk_d[:, 8:12])
            nc.sync.dma_start(wqg[3][:], wq_d[:, 12:16])
            nc.sync.dma_start(wkg[3][:], wk_d[:, 12:16])

            vall = persist.tile([P, MT, H * E], BF16, name="vall_sb")
            multiT = persist.tile([P, H, NDEV], BF16, name="multiT_sb")
            htail = pstail.tile([P, H], FP32, name="htail_ps")

            qts = {}
            kts = {}

            def make_proj(h):
                """Per-matmul emitters for head h's Q/K projections, to be
                interleaved into an attend's iterations so the PE's OOO window
                always holds chain-independent work. Returns (q_mms, k_mms,
                proj_fin); proj_fin emits the qt cast. The K psum tile is
                allocated lazily per m-chunk (m-chunk-outer, kd-inner) and
                cast as soon as its chunk finishes, so only one ps2 buffer is
                held at a time -- the other rotates with deferred-V tiles."""
                qt = work.tile([P, NDEV], BF16, tag="qt", name="qt")
                qps = psq.tile([P, NDEV], FP32, tag="psq", name="qps")
                kt = work.tile([P, N], BF16, tag="kt", name="kt")
                kstate = {}

                def q_mm(kd):
                    nc.tensor.matmul(
                        qps[:],
                        wqg[h // 4][:, h % 4, kd, :],
                        xT[:, kd, :],
                        start=(kd == 0),
                        stop=(kd == KD - 1),
                        skip_group_check=True,
                    )

                def k_mm(j):
                    i, kd = j // KD, j % KD
                    ms, ml = m_chunks[i]
                    if kd == 0:
                        kstate[i] = ps2.tile([P, 512], FP32, tag="ps512",
                                             name=f"kps{i}")
                    nc.tensor.matmul(
                        kstate[i][:, :ml],
                        wkg[h // 4][:, h % 4, kd, :],
                        encT[:, kd, ms : ms + ml],
                        start=(kd == 0),
                        stop=(kd == KD - 1),
                        skip_group_check=True,
                    )
                    if kd == KD - 1:
                        nc.vector.tensor_copy(
                            out=kt[:, ms : ms + ml], in_=kstate.pop(i)[:, :ml]
                        )

                def proj_fin():
                    nc.vector.tensor_copy(out=qt[:], in_=qps[:])
                    qts[h] = qt
                    kts[h] = kt

                q_mms = [(lambda kd: lambda: q_mm(kd))(kd) for kd in range(KD)]
                k_mms = [(lambda j: lambda: k_mm(j))(j) for j in range(2 * KD)]
                return q_mms, k_mms, proj_fin

            def emit_proj(h):
                # standalone proj for the first DEPTH heads (the rest are
                # interleaved into attend iterations)
                q_mms, k_mms, proj_fin = make_proj(h)
                for f in q_mms:
                    f()
                for f in k_mms:
                    f()
                proj_fin()

            def make_v_quarter(q):
                """Per-matmul emitters for the V projection of he-quarter q
                (heads 4q..4q+3): vall[m%P, mt, 512q:512(q+1)]. The psum tile
                allocates lazily per key-tile and casts on its last matmul,
                holding one ps2 buffer at a time."""
                state = {}

                def v_mm(j):
                    mt, kd = j // KD, j % KD
                    if kd == 0:
                        state[mt] = ps2.tile([P, 512], FP32, tag="ps512",
                                             name="vps")
                    nc.tensor.matmul(
                        state[mt][:],
                        encT[:, kd, mt * P : (mt + 1) * P],
                        wv[:, q, kd],
                        start=(kd == 0),
                        stop=(kd == KD - 1),
                        skip_group_check=True,
                    )
                    if kd == KD - 1:
                        nc.vector.tensor_copy(
                            out=vall[:, mt, q * 512 : (q + 1) * 512],
                            in_=state.pop(mt)[:],
                        )

                return [(lambda j: lambda: v_mm(j))(j) for j in range(MT * KD)]

            def emit_v_phase(quarters):
                for q in quarters:
                    for f in make_v_quarter(q):
                        f()

            def emit_attend(h, q_mms=(), k_mms=(), v_mms=(), fillers=()):
                # scores^T, softmax over free axis, headsT accum over key
                # tiles. The heads matmul is emitted DELAY iterations behind
                # the scores matmul (its stationary vsc comes off the softmax
                # chain ~2us later), and the next head's Q/K proj matmuls are
                # interleaved per-iteration so the PE's OOO exec window always
                # holds chain-independent work.
                DELAY = 3
                q_mms = list(q_mms)
                k_mms = list(k_mms)
                v_mms = list(v_mms)
                fillers = list(fillers)
                qt = qts.pop(h)
                kt = kts.pop(h)
                hps = psacc.tile([P, NDEV], FP32, tag="hacc", name="hps")
                abuf = {}
                vbuf = {}

                def emit_heads(mt):
                    nc.tensor.matmul(
                        hps[:],
                        vbuf[mt][:],
                        abuf.pop(mt)[:],
                        start=(mt == 0),
                        stop=(mt == MT - 1),
                        skip_group_check=True,
                    )
                    # tail output row: heads[512] column accumulates in a
                    # shared psum bank (read once after the last attend)
                    nc.tensor.matmul(
                        htail[:, h : h + 1],
                        vbuf.pop(mt)[:],
                        e512[:, mt, h : h + 1],
                        start=(mt == 0),
                        stop=(mt == MT - 1),
                        skip_group_check=True,
                    )

                for mt in range(MT):
                    tps = psnv.tile([P, NDEV], FP32, tag="psnv", name="tps")
                    nc.tensor.matmul(
                        tps[:],
                        kt[:, mt * P : (mt + 1) * P],
                        qt[:],
                        start=True,
                        stop=True,
                    )
                    a_sb = apool.tile([P, NDEV], BF16, tag="a", name="a_sb")
                    ssum = stats.tile([P, 1], FP32, tag="ssum", name="ssum")
                    nc.scalar.activation(
                        a_sb[:],
                        tps[:],
                        mybir.ActivationFunctionType.Exp,
                        scale=scale,
                        accum_out=ssum[:],
                    )
                    # denominators include the host tail-query column
                    ssumt = stats.tile([P, 1], FP32, tag="ssumt", name="ssumt")
                    nc.vector.tensor_tensor(
                        ssumt[:], ssum[:], e512[:, mt, h : h + 1],
                        mybir.AluOpType.add,
                    )
                    rcp = stats.tile([P, 1], FP32, tag="rcp", name="rcp")
                    nc.vector.reciprocal(rcp[:], ssumt[:])
                    vsc = apool.tile([P, E], BF16, tag="vsc", name="vsc")
                    nc.vector.tensor_scalar_mul(
                        vsc[:], vall[:, mt, h * E : (h + 1) * E], rcp[:]
                    )
                    abuf[mt] = a_sb
                    vbuf[mt] = vsc
                    if q_mms:
                        q_mms.pop(0)()
                    if mt >= DELAY:
                        emit_heads(mt - DELAY)
                    if k_mms:
                        k_mms.pop(0)()
                        k_mms.pop(0)()
                    if v_mms:
                        v_mms.pop(0)()
                        v_mms.pop(0)()
                    if fillers and mt % 2 == 1:
                        fillers.pop(0)()

                def finish():
                    # last DELAY heads matmuls + the multiT copy; the trailing
                    # chain latency hides under the next head's independent
                    # scores/proj matmuls via the PE OOO window.
                    for mt in range(MT - DELAY, MT):
                        emit_heads(mt)
                    nc.vector.tensor_copy(out=multiT[:, h, :], in_=hps[:])
                    for f in fillers:
                        f()

                return finish

            fin_parts = {}

            def emit_final_chunk(ns, nl, ds_, dl, half):
                # out[n, d] = concat_heads @ w_agg, split by head range: half 0
                # (heads < H1) stashes a bf16 partial, half 1 adds it on the
                # vector engine and streams the output tile.
                fps = ps2.tile([P, 512], FP32, tag="ps512", name="fps")
                hts = range(0, H1) if half == 0 else range(H1, H)
                for ht in hts:
                    nc.tensor.matmul(
                        fps[:nl, :dl],
                        multiT[:, ht, ns : ns + nl],
                        wagg[:, ht, ds_ : ds_ + dl],
                        start=(ht == hts[0]),
                        stop=(ht == hts[-1]),
                    )
                if half == 0:
                    part = fpool.tile(
                        [P, 512], BF16, tag=f"part{(ns // P) * 2 + ds_ // 512}",
                        name="part",
                    )
                    nc.vector.tensor_copy(out=part[:nl, :dl], in_=fps[:nl, :dl])
                    fin_parts[(ns, ds_)] = part
                else:
                    osb = opool.tile([P, 512], BF16, tag="osb", name="osb")
                    part = fin_parts.pop((ns, ds_))
                    nc.vector.tensor_tensor(
                        osb[:nl, :dl],
                        fps[:nl, :dl],
                        part[:nl, :dl],
                        mybir.AluOpType.add,
                    )
                    nc.sync.dma_start(out_d[ns : ns + nl, ds_ : ds_ + dl], osb[:nl, :dl])

            # Software pipeline: proj(h) runs DEPTH ahead of attend(h); the V
            # phase covers the encT/wv DMA stream. Each attend interleaves the
            # (h+DEPTH) head's proj matmuls per-iteration; trailing heads
            # matmuls are deferred into the next attend's start.
            for h in range(DEPTH):
                emit_proj(h)
            # V quarters 0-1 upfront (attends 0-7 read them); quarters 2-3
            # are deferred into attends 0-7 as an extra interleave stream so
            # the PE isn't gated on the tail of the wv DMA
            emit_v_phase([0, 1])
            vdef = make_v_quarter(2) + make_v_quarter(3)
            for h in range(DEPTH, H):
                q_mms, k_mms, proj_fin = make_proj(h)
                share, vdef = vdef[: 2 * MT], vdef[2 * MT :]
                fin_new = emit_attend(h - DEPTH, q_mms=q_mms, k_mms=k_mms,
                                      v_mms=share)
                proj_fin()
                fin_new()

            all_chunks = [
                (ns, nl, ds_, dl) for ns, nl in n_tiles for ds_, dl in d_chunks
            ]
            # Drain: the last DEPTH attends have no proj work left; interleave
            # final chunks over heads 0..H1-1 as PE filler.
            drain = list(range(H - DEPTH, H))
            per = (len(all_chunks) + len(drain) - 1) // len(drain)
            for i, h in enumerate(drain):
                cs = all_chunks[i * per : (i + 1) * per]
                fils = [(lambda c: lambda: emit_final_chunk(*c, 0))(c) for c in cs]
                fin = emit_attend(h, fillers=fils[:-1])
                fils[-1]()
                fin()
            # ship the tail heads column while the last final chunks run
            tailc = opool.tile([P, H], BF16, tag="tailc", name="tailc")
            nc.vector.tensor_copy(out=tailc[:], in_=htail[:])
            nc.gpsimd.dma_start(tail_d[:], tailc[:])
            for c in all_chunks:
                emit_final_chunk(*c, 1)

    nc.compile()
    return nc


def kernel(x, encoder_context, attention_mask, wq, wk, wv, w_agg, current_index):
    global LAST_RESULTS
    x = np.asarray(x)
    enc = np.asarray(encoder_context)
    wq = np.asarray(wq)
    wk = np.asarray(wk)
    wv = np.asarray(wv)
    w_agg = np.asarray(w_agg)
    ci = int(np.asarray(current_index))
    NV = min(ci + 1, N - 1)
    NDEV = NV - 1
    assert NV % P == 1 and NV > P, "kernel tuned for NV = k*128 + 1 (spec: 513)"

    nc = _cache.get(NV)
    if nc is None:
        nc = _build(NV)
        _cache[NV] = nc

    bf = ml_dtypes.bfloat16
    # weight layouts: see dram tensor declarations in _build
    wq_h = np.ascontiguousarray(wq.reshape(H, KD, P, E).transpose(2, 0, 1, 3)).astype(bf)
    wk_h = np.ascontiguousarray(wk.reshape(H, KD, P, E).transpose(2, 0, 1, 3)).astype(bf)
    wv_h = np.ascontiguousarray(
        wv.reshape(4, H // 4, KD, P, E).transpose(3, 0, 2, 1, 4)
    ).astype(bf)
    wagg_h = np.ascontiguousarray(w_agg.reshape(H, P, D).transpose(1, 0, 2)).astype(bf)

    scale = 1.0 / np.sqrt(np.float32(E))
    in_maps = []
    for b in range(B):
        xT_b = np.ascontiguousarray(
            x[b, :NDEV, :].T.reshape(KD, P, NDEV).transpose(1, 0, 2)
        ).astype(bf)
        encT_b = np.ascontiguousarray(
            enc[b].T.reshape(KD, P, N).transpose(1, 0, 2)
        ).astype(bf)
        # Tail-query score row, computed exactly on the host:
        #   q512[h] = x[512] @ wq[h];  s512[h, m] = enc[m] . (wk[h] @ q512[h])
        q512 = np.einsum("d,hde->he", x[b, NDEV], wq, optimize=True)
        t = np.einsum("hde,he->hd", wk, q512, optimize=True)
        s512 = enc[b].astype(np.float32) @ t.T.astype(np.float32)  # [M, H]
        e512_b = np.ascontiguousarray(
            np.exp(s512 * scale).reshape(MT, P, H).transpose(1, 0, 2)
        ).astype(bf)
        in_maps.append(
            {
                "xT": xT_b,
                "encT": encT_b,
                "wq": wq_h,
                "wk": wk_h,
                "wv": wv_h,
                "wagg": wagg_h,
                "e512": e512_b,
            }
        )

    if TRACE:
        _ensure_ntff_hook()
    res = run_bass_kernel_spmd(
        nc, in_maps, core_ids=list(range(NCORES)), trace=TRACE
    )
    LAST_RESULTS = res

    out = np.zeros((B, N, D), np.float32)
    wagg_f = w_agg.astype(np.float32)
    for b in range(B):
        r = res.results[b]
        out[b, :NDEV, :] = np.asarray(r["out"]).astype(np.float32)
        # tail_he[p, h] = heads[512, h*E + p]
        t = np.asarray(r["tail_he"]).astype(np.float32)
        out[b, NDEV, :] = t.T.reshape(H * E) @ wagg_f
    return out


# revision 58
# speedup vs baseline: 1.0140x; 1.0043x over previous
"""Trainium2 Bass kernel for nn_EncoderDecoderAttention (B=8, N=1024, D=1024, E=128, H=16).

Math (per batch b):
  Q = x @ wq[h]          [N, E]
  K = enc @ wk[h]        [N, E]
  V = enc @ wv[h]        [N, E]
  s = (Q K^T + mask) / sqrt(E)   with mask rows n >= NV set to -inf, NV = min(current_index+1, N-1)
  attn = softmax over the QUERY axis (per key column)
  heads = attn @ V; out = concat_heads @ w_agg

Because masked query rows are -inf before the softmax, attn rows n >= NV are exactly
zero, so output rows n >= NV are exactly zero: the device only computes rows [0, NV).

Sharding: pure data-parallel over batch across the 8 NeuronCores (one batch element
per core, full heads per core, no collectives).

Device layout (per core), NV = 513 fast path:
  The device computes queries 0..511 (every matmul FD=512-aligned). Query 512 only
  feeds (a) the softmax denominators and (b) output row 512; its unnormalized score
  row exp512[h, m] = exp(q512 . K_h[m] / sqrt(E)) is precomputed on the host
  (~0.3 GFLOP of glue) and shipped as a tiny input, so the ragged FD=1 matmuls for
  Q/scores disappear. Per (h, key-tile):
    scoresT = K^T-tile stationary x Q^T  -> psum [128, 512] (one bank)
    exp on scalar engine (fused free-axis accum) -> a_sb bf16 + ssum
    ssum += exp512 column; rcp = 1/ssum (vector); vsc = V-block * rcp
    headsT += vsc x a_sb  (+ FD=1 tail column from exp512 into a shared psum bank)
  The final w_agg matmul is split: heads 0-11 chunks are interleaved into the
  attend drain as PE filler, heads 12-15 finish after the last attend, adding the
  stashed partial on the vector engine; output streams out bf16 (host upcasts).
"""

import sys

if "/opt/trn_rl_repo" not in sys.path:
    sys.path.insert(0, "/opt/trn_rl_repo")

import ml_dtypes
import numpy as np

import concourse.mybir as mybir
import concourse.tile as tile
from concourse import bacc
from concourse.bass_utils import run_bass_kernel_spmd

B, N, D, E, H = 8, 1024, 1024, 128, 16
P = 128
KD = D // P  # contraction tiles over D
MT = N // P  # key tiles over N
NCORES = 8
BF16 = mybir.dt.bfloat16
FP32 = mybir.dt.float32

# test.py can flip these to profile
TRACE = False
LAST_RESULTS = None

_cache = {}


def _ensure_ntff_hook():
    """Register the axon NTFF profiling hook if the boot shim couldn't.

    Adapted from trn_agent_boot/trn_boot.py: the agent image's ``antenv``
    package lacks ``axon_hooks``, so ``trace=True`` silently skips NTFF
    capture. Inject an equivalent module backed by ctypes calls into the
    axon PJRT .so. Also neuter ``upload_artifacts`` (zero-egress box).
    """
    import contextlib
    import ctypes
    import os
    import types

    try:
        from antenv.axon_hooks import get_axon_ntff_profile_hook  # noqa: F401

        return
    except ImportError:
        pass

    so_path = "/opt/axon/libaxon_pjrt.so"
    if not os.path.exists(so_path):
        return
    lib = ctypes.CDLL(so_path)
    if not hasattr(lib, "axon_start_nrt_profile"):
        return
    lib.axon_start_nrt_profile.argtypes = [
        ctypes.POINTER(ctypes.c_int64),
        ctypes.c_size_t,
    ]
    lib.axon_start_nrt_profile.restype = ctypes.c_int64
    lib.axon_stop_nrt_profile.argtypes = [ctypes.c_char_p]
    lib.axon_stop_nrt_profile.restype = ctypes.c_int64

    @contextlib.contextmanager
    def _hook(output_dir, device_ids):
        import jax

        jax.devices()
        if device_ids:
            ids = (ctypes.c_int64 * len(device_ids))(*device_ids)
            rc = lib.axon_start_nrt_profile(ids, len(device_ids))
        else:
            rc = lib.axon_start_nrt_profile(None, 0)
        if rc != 0:
            raise RuntimeError(f"axon_start_nrt_profile rc={rc}")
        try:
            yield
        finally:
            n = lib.axon_stop_nrt_profile(str(output_dir).encode())
            print(f"ntff profile: {n} file(s) -> {output_dir}", file=sys.stderr)

    mod = types.ModuleType("antenv.axon_hooks")
    mod.get_axon_ntff_profile_hook = lambda: _hook
    mod.set_axon_ntff_profile_hook = lambda h: None
    sys.modules["antenv.axon_hooks"] = mod

    # upload_artifacts reaches for a bucket; keep everything local.
    from concourse import bass_utils as _bu

    _orig_upload = _bu.upload_artifacts

    def _safe_upload(tmpdir):
        try:
            return _orig_upload(tmpdir)
        except Exception:
            return str(tmpdir)

    _bu.upload_artifacts = _safe_upload


def _chunks(total, step):
    return [(s, min(step, total - s)) for s in range(0, total, step)]


def _build(NV):
    """Fast path for NV = k*128 + 1 (the shipped case: NV=513)."""
    NDEV = NV - 1  # device-computed query rows, tile-aligned
    nc = bacc.Bacc("TRN2", target_bir_lowering=False, debug=False, num_devices=NCORES)

    xT_d = nc.dram_tensor("xT", [P, KD, NDEV], BF16, kind="ExternalInput")
    encT_d = nc.dram_tensor("encT", [P, KD, N], BF16, kind="ExternalInput")
    wq_d = nc.dram_tensor("wq", [P, H, KD, E], BF16, kind="ExternalInput")
    wk_d = nc.dram_tensor("wk", [P, H, KD, E], BF16, kind="ExternalInput")
    wv_d = nc.dram_tensor("wv", [P, 4, KD, H // 4, E], BF16, kind="ExternalInput")
    wagg_d = nc.dram_tensor("wagg", [P, H, D], BF16, kind="ExternalInput")
    # exp of the tail query's score row, keys on partitions: [m%P, mt, h]
    e512_d = nc.dram_tensor("e512", [P, MT, H], BF16, kind="ExternalInput")
    out_d = nc.dram_tensor("out", [NDEV, D], BF16, kind="ExternalOutput")
    tail_d = nc.dram_tensor("tail_he", [P, H], BF16, kind="ExternalOutput")

    n_tiles = _chunks(NDEV, P)
    he_chunks = _chunks(H * E, 512)
    d_chunks = _chunks(D, 512)
    m_chunks = _chunks(N, 512)
    scale = 1.0 / float(np.sqrt(E))

    DEPTH = 2
    H1 = 12  # final-phase heads computed as drain filler; H-H1 finish at the end

    with tile.TileContext(nc) as tc:
        with (
            tc.tile_pool(name="persist", bufs=1) as persist,
            tc.tile_pool(name="wgroup", bufs=2) as wgroup,
            tc.tile_pool(name="work", bufs=3) as work,
            tc.tile_pool(name="apool", bufs=6) as apool,
            tc.tile_pool(name="stats", bufs=8) as stats,
            tc.tile_pool(name="opool", bufs=3) as opool,
            tc.tile_pool(name="fpool", bufs=1) as fpool,
            tc.tile_pool(name="ps2", bufs=2, space="PSUM") as ps2,
            tc.tile_pool(name="psnv", bufs=3, space="PSUM") as psnv,
            tc.tile_pool(name="psq", bufs=1, space="PSUM") as psq,
            tc.tile_pool(name="psacc", bufs=1, space="PSUM") as psacc,
            tc.tile_pool(name="pstail", bufs=1, space="PSUM") as pstail,
        ):
            # Warm the PE clock gate ASAP with a short dependency-light dummy
            # burst (the PE queue is in-order, so a long burst would delay the
            # first real matmuls instead).
            scratch = persist.tile([P, 256], BF16, name="warm_scratch")
            nc.vector.memset(scratch[:], 0.0)
            dpsA = ps2.tile([P, 512], FP32, tag="ps512", name="dpsA")
            dpsB = ps2.tile([P, 512], FP32, tag="ps512", name="dpsB")
            for i in range(8):
                nc.tensor.matmul(
                    (dpsA if i % 2 == 0 else dpsB)[:, :256],
                    scratch[:, :P],
                    scratch[:],
                    start=True,
                    stop=True,
                    skip_group_check=True,
                )

            # DMA issues serialize at ~700ns each on the queue, so use FEW,
            # LARGE transfers (host pre-lays everything out contiguously),
            # ordered by on-device deadline.
            xT = persist.tile([P, KD, NDEV], BF16, name="xT_sb")
            encT = persist.tile([P, KD, N], BF16, name="encT_sb")
            e512 = persist.tile([P, MT, H], BF16, name="e512_sb")
            wv = persist.tile([P, 4, KD, H // 4, E], BF16, name="wv_sb")
            wagg = persist.tile([P, H, D], BF16, name="wagg_sb")
            # wq/wk stream through rotating 4-head group tiles (one DMA issue
            # per group; a group's issue stalls the queue until its buffer
            # frees, so late groups are ordered after everything early)
            wqg = [wgroup.tile([P, 4, KD, E], BF16, tag="wq", name=f"wqg{g}")
                   for g in range(4)]
            wkg = [wgroup.tile([P, 4, KD, E], BF16, tag="wk", name=f"wkg{g}")
                   for g in range(4)]

            nc.sync.dma_start(wqg[0][:, 0:1], wq_d[:, 0:1])
            nc.sync.dma_start(xT[:, 0:2, :], xT_d[:, 0:2, :])
            nc.sync.dma_start(xT[:, 2:4, :], xT_d[:, 2:4, :])
            nc.sync.dma_start(xT[:, 4:6, :], xT_d[:, 4:6, :])
            nc.sync.dma_start(xT[:, 6:KD, :], xT_d[:, 6:KD, :])
            nc.sync.dma_start(wkg[0][:, 0:1], wk_d[:, 0:1])
            nc.sync.dma_start(encT[:, 0:2, :], encT_d[:, 0:2, :])
            nc.sync.dma_start(encT[:, 2:4, :], encT_d[:, 2:4, :])
            nc.sync.dma_start(encT[:, 4:6, :], encT_d[:, 4:6, :])
            nc.sync.dma_start(encT[:, 6:KD, :], encT_d[:, 6:KD, :])
            nc.sync.dma_start(wqg[0][:, 1:4], wq_d[:, 1:4])
            nc.sync.dma_start(wkg[0][:, 1:4], wk_d[:, 1:4])
            nc.sync.dma_start(e512[:], e512_d[:])
            nc.sync.dma_start(wqg[1][:], wq_d[:, 4:8])
            nc.sync.dma_start(wkg[1][:], wk_d[:, 4:8])
            nc.sync.dma_start(wv[:, 0], wv_d[:, 0])
            nc.sync.dma_start(wv[:, 1], wv_d[:, 1])
            nc.sync.dma_start(wv[:, 2], wv_d[:, 2])
            nc.sync.dma_start(wv[:, 3], wv_d[:, 3])
            nc.sync.dma_start(wagg[:], wagg_d[:])
            nc.sync.dma_start(wqg[2][:], wq_d[:, 8:12])
            nc.sync.dma_start(wkg[2][:], wk_d[:, 8:12])
            nc.sync.dma_start(wqg[3][:], wq_d[:, 12:16])
            nc.sync.dma_start(wkg[3][:], wk_d[:, 12:16])

            vall = persist.tile([P, MT, H * E], BF16, name="vall_sb")
            multiT = persist.tile([P, H, NDEV], BF16, name="multiT_sb")
            htail = pstail.tile([P, H], FP32, name="htail_ps")

            qts = {}
            kts = {}

            def make_proj(h):
                """Per-matmul emitters for head h's Q/K projections, to be
                interleaved into an attend's iterations so the PE's OOO window
                always holds chain-independent work. Returns (q_mms, k_mms,
                proj_fin); proj_fin emits the qt cast. The K psum tile is
                allocated lazily per m-chunk (m-chunk-outer, kd-inner) and
                cast as soon as its chunk finishes, so only one ps2 buffer is
                held at a time -- the other rotates with deferred-V tiles."""
                qt = work.tile([P, NDEV], BF16, tag="qt", name="qt")
                qps = psq.tile([P, NDEV], FP32, tag="psq", name="qps")
                kt = work.tile([P, N], BF16, tag="kt", name="kt")
                kstate = {}

                def q_mm(kd):
                    nc.tensor.matmul(
                        qps[:],
                        wqg[h // 4][:, h % 4, kd, :],
                        xT[:, kd, :],
                        start=(kd == 0),
                        stop=(kd == KD - 1),
                        skip_group_check=True,
                    )

                def k_mm(j):
                    i, kd = j // KD, j % KD
                    ms, ml = m_chunks[i]
                    if kd == 0:
                        kstate[i] = ps2.tile([P, 512], FP32, tag="ps512",
                                             name=f"kps{i}")
                    nc.tensor.matmul(
                        kstate[i][:, :ml],
                        wkg[h // 4][:, h % 4, kd, :],
                        encT[:, kd, ms : ms + ml],
                        start=(kd == 0),
                        stop=(kd == KD - 1),
                        skip_group_check=True,
                    )
                    if kd == KD - 1:
                        nc.vector.tensor_copy(
                            out=kt[:, ms : ms + ml], in_=kstate.pop(i)[:, :ml]
                        )

                def proj_fin():
                    nc.vector.tensor_copy(out=qt[:], in_=qps[:])
                    qts[h] = qt
                    kts[h] = kt

                q_mms = [(lambda kd: lambda: q_mm(kd))(kd) for kd in range(KD)]
                k_mms = [(lambda j: lambda: k_mm(j))(j) for j in range(2 * KD)]
                return q_mms, k_mms, proj_fin

            def emit_proj(h):
                # standalone proj for the first DEPTH heads (the rest are
                # interleaved into attend iterations)
                q_mms, k_mms, proj_fin = make_proj(h)
                for f in q_mms:
                    f()
                for f in k_mms:
                    f()
                proj_fin()

            def make_v_quarter(q):
                """Per-matmul emitters for the V projection of he-quarter q
                (heads 4q..4q+3): vall[m%P, mt, 512q:512(q+1)]. The psum tile
                allocates lazily per key-tile and casts on its last matmul,
                holding one ps2 buffer at a time."""
                state = {}

                def v_mm(j):
                    mt, kd = j // KD, j % KD
                    if kd == 0:
                        state[mt] = ps2.tile([P, 512], FP32, tag="ps512",
                                             name="vps")
                    nc.tensor.matmul(
                        state[mt][:],
                        encT[:, kd, mt * P : (mt + 1) * P],
                        wv[:, q, kd],
                        start=(kd == 0),
                        stop=(kd == KD - 1),
                        skip_group_check=True,
                    )
                    if kd == KD - 1:
                        nc.vector.tensor_copy(
                            out=vall[:, mt, q * 512 : (q + 1) * 512],
                            in_=state.pop(mt)[:],
                        )

                return [(lambda j: lambda: v_mm(j))(j) for j in range(MT * KD)]

            def emit_v_phase(quarters):
                for q in quarters:
                    for f in make_v_quarter(q):
                        f()

            def emit_attend(h, q_mms=(), k_mms=(), v_mms=(), fillers=()):
                # scores^T, softmax over free axis, headsT accum over key
                # tiles. The heads matmul is emitted DELAY iterations behind
                # the scores matmul (its stationary vsc comes off the softmax
                # chain ~2us later), and the next head's Q/K proj matmuls are
                # interleaved per-iteration so the PE's OOO exec window always
                # holds chain-independent work.
                DELAY = 4
                q_mms = list(q_mms)
                k_mms = list(k_mms)
                v_mms = list(v_mms)
                fillers = list(fillers)
                qt = qts.pop(h)
                kt = kts.pop(h)
                hps = psacc.tile([P, NDEV], FP32, tag="hacc", name="hps")
                abuf = {}
                vbuf = {}

                def emit_heads(mt):
                    nc.tensor.matmul(
                        hps[:],
                        vbuf[mt][:],
                        abuf.pop(mt)[:],
                        start=(mt == 0),
                        stop=(mt == MT - 1),
                        skip_group_check=True,
                    )
                    # tail output row: heads[512] column accumulates in a
                    # shared psum bank (read once after the last attend)
                    nc.tensor.matmul(
                        htail[:, h : h + 1],
                        vbuf.pop(mt)[:],
                        e512[:, mt, h : h + 1],
                        start=(mt == 0),
                        stop=(mt == MT - 1),
                        skip_group_check=True,
                    )

                for mt in range(MT):
                    tps = psnv.tile([P, NDEV], FP32, tag="psnv", name="tps")
                    nc.tensor.matmul(
                        tps[:],
                        kt[:, mt * P : (mt + 1) * P],
                        qt[:],
                        start=True,
                        stop=True,
                    )
                    a_sb = apool.tile([P, NDEV], BF16, tag="a", name="a_sb")
                    ssum = stats.tile([P, 1], FP32, tag="ssum", name="ssum")
                    nc.scalar.activation(
                        a_sb[:],
                        tps[:],
                        mybir.ActivationFunctionType.Exp,
                        scale=scale,
                        accum_out=ssum[:],
                    )
                    # denominators include the host tail-query column
                    ssumt = stats.tile([P, 1], FP32, tag="ssumt", name="ssumt")
                    nc.vector.tensor_tensor(
                        ssumt[:], ssum[:], e512[:, mt, h : h + 1],
                        mybir.AluOpType.add,
                    )
                    rcp = stats.tile([P, 1], FP32, tag="rcp", name="rcp")
                    nc.vector.reciprocal(rcp[:], ssumt[:])
                    vsc = apool.tile([P, E], BF16, tag="vsc", name="vsc")
                    nc.vector.tensor_scalar_mul(
                        vsc[:], vall[:, mt, h * E : (h + 1) * E], rcp[:]
                    )
                    abuf[mt] = a_sb
                    vbuf[mt] = vsc
                    if q_mms:
                        q_mms.pop(0)()
                    if mt >= DELAY:
                        emit_heads(mt - DELAY)
                    if k_mms:
                        k_mms.pop(0)()
                        k_mms.pop(0)()
                    if v_mms:
                        v_mms.pop(0)()
                        v_mms.pop(0)()
                    if fillers and mt % 2 == 1:
                        fillers.pop(0)()

                def finish():
                    # last DELAY heads matmuls + the multiT copy; the trailing
                    # chain latency hides under the next head's independent
                    # scores/proj matmuls via the PE OOO window.
                    for mt in range(MT - DELAY, MT):
                        emit_heads(mt)
                    nc.vector.tensor_copy(out=multiT[:, h, :], in_=hps[:])
                    for f in fillers:
                        f()

                return finish

            fin_parts = {}

            def emit_final_chunk(ns, nl, ds_, dl, half):
                # out[n, d] = concat_heads @ w_agg, split by head range: half 0
                # (heads < H1) stashes a bf16 partial, half 1 adds it on the
                # vector engine and streams the output tile.
                fps = ps2.tile([P, 512], FP32, tag="ps512", name="fps")
                hts = range(0, H1) if half == 0 else range(H1, H)
                for ht in hts:
                    nc.tensor.matmul(
                        fps[:nl, :dl],
                        multiT[:, ht, ns : ns + nl],
                        wagg[:, ht, ds_ : ds_ + dl],
                        start=(ht == hts[0]),
                        stop=(ht == hts[-1]),
                    )
                if half == 0:
                    part = fpool.tile(
                        [P, 512], BF16, tag=f"part{(ns // P) * 2 + ds_ // 512}",
                        name="part",
                    )
                    nc.vector.tensor_copy(out=part[:nl, :dl], in_=fps[:nl, :dl])
                    fin_parts[(ns, ds_)] = part
                else:
                    osb = opool.tile([P, 512], BF16, tag="osb", name="osb")
                    part = fin_parts.pop((ns, ds_))
                    nc.vector.tensor_tensor(
                        osb[:nl, :dl],
                        fps[:nl, :dl],
                        part[:nl, :dl],
                        mybir.AluOpType.add,
                    )
                    nc.sync.dma_start(out_d[ns : ns + nl, ds_ : ds_ + dl], osb[:nl, :dl])

            # Software pipeline: proj(h) runs DEPTH ahead of attend(h); the V
            # phase covers the encT/wv DMA stream. Each attend interleaves the
            # (h+DEPTH) head's proj matmuls per-iteration; trailing heads
            # matmuls are deferred into the next attend's start.
            for h in range(DEPTH):
                emit_proj(h)
            # V quarters 0-1 upfront (attends 0-7 read them); quarters 2-3
            # are deferred into attends 0-7 as an extra interleave stream so
            # the PE isn't gated on the tail of the wv DMA
            emit_v_phase([0, 1])
            vdef = make_v_quarter(2) + make_v_quarter(3)
            for h in range(DEPTH, H):
                q_mms, k_mms, proj_fin = make_proj(h)
                share, vdef = vdef[: 2 * MT], vdef[2 * MT :]
                fin_new = emit_attend(h - DEPTH, q_mms=q_mms, k_mms=k_mms,
                                      v_mms=share)
                proj_fin()
                fin_new()

            all_chunks = [
                (ns, nl, ds_, dl) for ns, nl in n_tiles for ds_, dl in d_chunks
            ]
            # Drain: the last DEPTH attends have no proj work left; interleave
            # final chunks over heads 0..H1-1 as PE filler.
            drain = list(range(H - DEPTH, H))
            per = (len(all_chunks) + len(drain) - 1) // len(drain)
            for i, h in enumerate(drain):
                cs = all_chunks[i * per : (i + 1) * per]
                fils = [(lambda c: lambda: emit_final_chunk(*c, 0))(c) for c in cs]
                fin = emit_attend(h, fillers=fils[:-1])
                fils[-1]()
                fin()
            # ship the tail heads column while the last final chunks run
            tailc = opool.tile([P, H], BF16, tag="tailc", name="tailc")
            nc.vector.tensor_copy(out=tailc[:], in_=htail[:])
            nc.gpsimd.dma_start(tail_d[:], tailc[:])
            for c in all_chunks:
                emit_final_chunk(*c, 1)

    nc.compile()
    return nc


def kernel(x, encoder_context, attention_mask, wq, wk, wv, w_agg, current_index):
    global LAST_RESULTS
    x = np.asarray(x)
    enc = np.asarray(encoder_context)
    wq = np.asarray(wq)
    wk = np.asarray(wk)
    wv = np.asarray(wv)
    w_agg = np.asarray(w_agg)
    ci = int(np.asarray(current_index))
    NV = min(ci + 1, N - 1)
    NDEV = NV - 1
    assert NV % P == 1 and NV > P, "kernel tuned for NV = k*128 + 1 (spec: 513)"

    nc = _cache.get(NV)
    if nc is None:
        nc = _build(NV)
        _cache[NV] = nc

    bf = ml_dtypes.bfloat16
    # weight layouts: see dram tensor declarations in _build
    wq_h = np.ascontiguousarray(wq.reshape(H, KD, P, E).transpose(2, 0, 1, 3)).astype(bf)
    wk_h = np.ascontiguousarray(wk.reshape(H, KD, P, E).transpose(2, 0, 1, 3)).astype(bf)
    wv_h = np.ascontiguousarray(
        wv.reshape(4, H // 4, KD, P, E).transpose(3, 0, 2, 1, 4)
    ).astype(bf)
    wagg_h = np.ascontiguousarray(w_agg.reshape(H, P, D).transpose(1, 0, 2)).astype(bf)

    scale = 1.0 / np.sqrt(np.float32(E))
    in_maps = []
    for b in range(B):
        xT_b = np.ascontiguousarray(
            x[b, :NDEV, :].T.reshape(KD, P, NDEV).transpose(1, 0, 2)
        ).astype(bf)
        encT_b = np.ascontiguousarray(
            enc[b].T.reshape(KD, P, N).transpose(1, 0, 2)
        ).astype(bf)
        # Tail-query score row, computed exactly on the host:
        #   q512[h] = x[512] @ wq[h];  s512[h, m] = enc[m] . (wk[h] @ q512[h])
        q512 = np.einsum("d,hde->he", x[b, NDEV], wq, optimize=True)
        t = np.einsum("hde,he->hd", wk, q512, optimize=True)
        s512 = enc[b].astype(np.float32) @ t.T.astype(np.float32)  # [M, H]
        e512_b = np.ascontiguousarray(
            np.exp(s512 * scale).reshape(MT, P, H).transpose(1, 0, 2)
        ).astype(bf)
        in_maps.append(
            {
                "xT": xT_b,
                "encT": encT_b,
                "wq": wq_h,
                "wk": wk_h,
                "wv": wv_h,
                "wagg": wagg_h,
                "e512": e512_b,
            }
        )

    if TRACE:
        _ensure_ntff_hook()
    res = run_bass_kernel_spmd(
        nc, in_maps, core_ids=list(range(NCORES)), trace=TRACE
    )
    LAST_RESULTS = res

    out = np.zeros((B, N, D), np.float32)
    wagg_f = w_agg.astype(np.float32)
    for b in range(B):
        r = res.results[b]
        out[b, :NDEV, :] = np.asarray(r["out"]).astype(np.float32)
        # tail_he[p, h] = heads[512, h*E + p]
        t = np.asarray(r["tail_he"]).astype(np.float32)
        out[b, NDEV, :] = t.T.reshape(H * E) @ wagg_f
    return out


# revision 59
# speedup vs baseline: 1.0190x; 1.0049x over previous
"""Trainium2 Bass kernel for nn_EncoderDecoderAttention (B=8, N=1024, D=1024, E=128, H=16).

Math (per batch b):
  Q = x @ wq[h]          [N, E]
  K = enc @ wk[h]        [N, E]
  V = enc @ wv[h]        [N, E]
  s = (Q K^T + mask) / sqrt(E)   with mask rows n >= NV set to -inf, NV = min(current_index+1, N-1)
  attn = softmax over the QUERY axis (per key column)
  heads = attn @ V; out = concat_heads @ w_agg

Because masked query rows are -inf before the softmax, attn rows n >= NV are exactly
zero, so output rows n >= NV are exactly zero: the device only computes rows [0, NV).

Sharding: pure data-parallel over batch across the 8 NeuronCores (one batch element
per core, full heads per core, no collectives).

Device layout (per core), NV = 513 fast path:
  The device computes queries 0..511 (every matmul FD=512-aligned). Query 512 only
  feeds (a) the softmax denominators and (b) output row 512; its unnormalized score
  row exp512[h, m] = exp(q512 . K_h[m] / sqrt(E)) is precomputed on the host
  (~0.3 GFLOP of glue) and shipped as a tiny input, so the ragged FD=1 matmuls for
  Q/scores disappear. Per (h, key-tile):
    scoresT = K^T-tile stationary x Q^T  -> psum [128, 512] (one bank)
    exp on scalar engine (fused free-axis accum) -> a_sb bf16 + ssum
    ssum += exp512 column; rcp = 1/ssum (vector); vsc = V-block * rcp
    headsT += vsc x a_sb  (+ FD=1 tail column from exp512 into a shared psum bank)
  The final w_agg matmul is split: heads 0-11 chunks are interleaved into the
  attend drain as PE filler, heads 12-15 finish after the last attend, adding the
  stashed partial on the vector engine; output streams out bf16 (host upcasts).
"""

import sys

if "/opt/trn_rl_repo" not in sys.path:
    sys.path.insert(0, "/opt/trn_rl_repo")

import ml_dtypes
import numpy as np

import concourse.mybir as mybir
import concourse.tile as tile
from concourse import bacc
from concourse.bass_utils import run_bass_kernel_spmd

B, N, D, E, H = 8, 1024, 1024, 128, 16
P = 128
KD = D // P  # contraction tiles over D
MT = N // P  # key tiles over N
NCORES = 8
BF16 = mybir.dt.bfloat16
FP32 = mybir.dt.float32

# test.py can flip these to profile
TRACE = False
LAST_RESULTS = None

_cache = {}


def _ensure_ntff_hook():
    """Register the axon NTFF profiling hook if the boot shim couldn't.

    Adapted from trn_agent_boot/trn_boot.py: the agent image's ``antenv``
    package lacks ``axon_hooks``, so ``trace=True`` silently skips NTFF
    capture. Inject an equivalent module backed by ctypes calls into the
    axon PJRT .so. Also neuter ``upload_artifacts`` (zero-egress box).
    """
    import contextlib
    import ctypes
    import os
    import types

    try:
        from antenv.axon_hooks import get_axon_ntff_profile_hook  # noqa: F401

        return
    except ImportError:
        pass

    so_path = "/opt/axon/libaxon_pjrt.so"
    if not os.path.exists(so_path):
        return
    lib = ctypes.CDLL(so_path)
    if not hasattr(lib, "axon_start_nrt_profile"):
        return
    lib.axon_start_nrt_profile.argtypes = [
        ctypes.POINTER(ctypes.c_int64),
        ctypes.c_size_t,
    ]
    lib.axon_start_nrt_profile.restype = ctypes.c_int64
    lib.axon_stop_nrt_profile.argtypes = [ctypes.c_char_p]
    lib.axon_stop_nrt_profile.restype = ctypes.c_int64

    @contextlib.contextmanager
    def _hook(output_dir, device_ids):
        import jax

        jax.devices()
        if device_ids:
            ids = (ctypes.c_int64 * len(device_ids))(*device_ids)
            rc = lib.axon_start_nrt_profile(ids, len(device_ids))
        else:
            rc = lib.axon_start_nrt_profile(None, 0)
        if rc != 0:
            raise RuntimeError(f"axon_start_nrt_profile rc={rc}")
        try:
            yield
        finally:
            n = lib.axon_stop_nrt_profile(str(output_dir).encode())
            print(f"ntff profile: {n} file(s) -> {output_dir}", file=sys.stderr)

    mod = types.ModuleType("antenv.axon_hooks")
    mod.get_axon_ntff_profile_hook = lambda: _hook
    mod.set_axon_ntff_profile_hook = lambda h: None
    sys.modules["antenv.axon_hooks"] = mod

    # upload_artifacts reaches for a bucket; keep everything local.
    from concourse import bass_utils as _bu

    _orig_upload = _bu.upload_artifacts

    def _safe_upload(tmpdir):
        try:
            return _orig_upload(tmpdir)
        except Exception:
            return str(tmpdir)

    _bu.upload_artifacts = _safe_upload


def _chunks(total, step):
    return [(s, min(step, total - s)) for s in range(0, total, step)]


def _build(NV):
    """Fast path for NV = k*128 + 1 (the shipped case: NV=513)."""
    NDEV = NV - 1  # device-computed query rows, tile-aligned
    nc = bacc.Bacc("TRN2", target_bir_lowering=False, debug=False, num_devices=NCORES)

    xT_d = nc.dram_tensor("xT", [P, KD, NDEV], BF16, kind="ExternalInput")
    encT_d = nc.dram_tensor("encT", [P, KD, N], BF16, kind="ExternalInput")
    wq_d = nc.dram_tensor("wq", [P, H, KD, E], BF16, kind="ExternalInput")
    wk_d = nc.dram_tensor("wk", [P, H, KD, E], BF16, kind="ExternalInput")
    wv_d = nc.dram_tensor("wv", [P, 4, KD, H // 4, E], BF16, kind="ExternalInput")
    wagg_d = nc.dram_tensor("wagg", [P, H, D], BF16, kind="ExternalInput")
    # exp of the tail query's score row, keys on partitions: [m%P, mt, h]
    e512_d = nc.dram_tensor("e512", [P, MT, H], BF16, kind="ExternalInput")
    out_d = nc.dram_tensor("out", [NDEV, D], BF16, kind="ExternalOutput")
    tail_d = nc.dram_tensor("tail_he", [P, H], BF16, kind="ExternalOutput")

    n_tiles = _chunks(NDEV, P)
    he_chunks = _chunks(H * E, 512)
    d_chunks = _chunks(D, 512)
    m_chunks = _chunks(N, 512)
    scale = 1.0 / float(np.sqrt(E))

    DEPTH = 2
    H1 = 12  # final-phase heads computed as drain filler; H-H1 finish at the end

    with tile.TileContext(nc) as tc:
        with (
            tc.tile_pool(name="persist", bufs=1) as persist,
            tc.tile_pool(name="wgroup", bufs=2) as wgroup,
            tc.tile_pool(name="work", bufs=3) as work,
            tc.tile_pool(name="apool", bufs=7) as apool,
            tc.tile_pool(name="stats", bufs=8) as stats,
            tc.tile_pool(name="opool", bufs=3) as opool,
            tc.tile_pool(name="fpool", bufs=1) as fpool,
            tc.tile_pool(name="ps2", bufs=2, space="PSUM") as ps2,
            tc.tile_pool(name="psnv", bufs=3, space="PSUM") as psnv,
            tc.tile_pool(name="psq", bufs=1, space="PSUM") as psq,
            tc.tile_pool(name="psacc", bufs=1, space="PSUM") as psacc,
            tc.tile_pool(name="pstail", bufs=1, space="PSUM") as pstail,
        ):
            # Warm the PE clock gate ASAP with a short dependency-light dummy
            # burst (the PE queue is in-order, so a long burst would delay the
            # first real matmuls instead).
            scratch = persist.tile([P, 256], BF16, name="warm_scratch")
            nc.vector.memset(scratch[:], 0.0)
            dpsA = ps2.tile([P, 512], FP32, tag="ps512", name="dpsA")
            dpsB = ps2.tile([P, 512], FP32, tag="ps512", name="dpsB")
            for i in range(8):
                nc.tensor.matmul(
                    (dpsA if i % 2 == 0 else dpsB)[:, :256],
                    scratch[:, :P],
                    scratch[:],
                    start=True,
                    stop=True,
                    skip_group_check=True,
                )

            # DMA issues serialize at ~700ns each on the queue, so use FEW,
            # LARGE transfers (host pre-lays everything out contiguously),
            # ordered by on-device deadline.
            xT = persist.tile([P, KD, NDEV], BF16, name="xT_sb")
            encT = persist.tile([P, KD, N], BF16, name="encT_sb")
            e512 = persist.tile([P, MT, H], BF16, name="e512_sb")
            wv = persist.tile([P, 4, KD, H // 4, E], BF16, name="wv_sb")
            wagg = persist.tile([P, H, D], BF16, name="wagg_sb")
            # wq/wk stream through rotating 4-head group tiles (one DMA issue
            # per group; a group's issue stalls the queue until its buffer
            # frees, so late groups are ordered after everything early)
            wqg = [wgroup.tile([P, 4, KD, E], BF16, tag="wq", name=f"wqg{g}")
                   for g in range(4)]
            wkg = [wgroup.tile([P, 4, KD, E], BF16, tag="wk", name=f"wkg{g}")
                   for g in range(4)]

            nc.sync.dma_start(wqg[0][:, 0:1], wq_d[:, 0:1])
            nc.sync.dma_start(xT[:, 0:2, :], xT_d[:, 0:2, :])
            nc.sync.dma_start(xT[:, 2:4, :], xT_d[:, 2:4, :])
            nc.sync.dma_start(xT[:, 4:6, :], xT_d[:, 4:6, :])
            nc.sync.dma_start(xT[:, 6:KD, :], xT_d[:, 6:KD, :])
            nc.sync.dma_start(wkg[0][:, 0:1], wk_d[:, 0:1])
            nc.sync.dma_start(encT[:, 0:2, :], encT_d[:, 0:2, :])
            nc.sync.dma_start(encT[:, 2:4, :], encT_d[:, 2:4, :])
            nc.sync.dma_start(encT[:, 4:6, :], encT_d[:, 4:6, :])
            nc.sync.dma_start(encT[:, 6:KD, :], encT_d[:, 6:KD, :])
            nc.sync.dma_start(wqg[0][:, 1:4], wq_d[:, 1:4])
            nc.sync.dma_start(wkg[0][:, 1:4], wk_d[:, 1:4])
            nc.sync.dma_start(e512[:], e512_d[:])
            nc.sync.dma_start(wqg[1][:], wq_d[:, 4:8])
            nc.sync.dma_start(wkg[1][:], wk_d[:, 4:8])
            nc.sync.dma_start(wv[:, 0], wv_d[:, 0])
            nc.sync.dma_start(wv[:, 1], wv_d[:, 1])
            nc.sync.dma_start(wv[:, 2], wv_d[:, 2])
            nc.sync.dma_start(wv[:, 3], wv_d[:, 3])
            nc.sync.dma_start(wagg[:], wagg_d[:])
            nc.sync.dma_start(wqg[2][:], wq_d[:, 8:12])
            nc.sync.dma_start(wkg[2][:], wk_d[:, 8:12])
            nc.sync.dma_start(wqg[3][:], wq_d[:, 12:16])
            nc.sync.dma_start(wkg[3][:], wk_d[:, 12:16])

            vall = persist.tile([P, MT, H * E], BF16, name="vall_sb")
            multiT = persist.tile([P, H, NDEV], BF16, name="multiT_sb")
            htail = pstail.tile([P, H], FP32, name="htail_ps")

            qts = {}
            kts = {}

            def make_proj(h):
                """Per-matmul emitters for head h's Q/K projections, to be
                interleaved into an attend's iterations so the PE's OOO window
                always holds chain-independent work. Returns (q_mms, k_mms,
                proj_fin); proj_fin emits the qt cast. The K psum tile is
                allocated lazily per m-chunk (m-chunk-outer, kd-inner) and
                cast as soon as its chunk finishes, so only one ps2 buffer is
                held at a time -- the other rotates with deferred-V tiles."""
                qt = work.tile([P, NDEV], BF16, tag="qt", name="qt")
                qps = psq.tile([P, NDEV], FP32, tag="psq", name="qps")
                kt = work.tile([P, N], BF16, tag="kt", name="kt")
                kstate = {}

                def q_mm(kd):
                    nc.tensor.matmul(
                        qps[:],
                        wqg[h // 4][:, h % 4, kd, :],
                        xT[:, kd, :],
                        start=(kd == 0),
                        stop=(kd == KD - 1),
                        skip_group_check=True,
                    )

                def k_mm(j):
                    i, kd = j // KD, j % KD
                    ms, ml = m_chunks[i]
                    if kd == 0:
                        kstate[i] = ps2.tile([P, 512], FP32, tag="ps512",
                                             name=f"kps{i}")
                    nc.tensor.matmul(
                        kstate[i][:, :ml],
                        wkg[h // 4][:, h % 4, kd, :],
                        encT[:, kd, ms : ms + ml],
                        start=(kd == 0),
                        stop=(kd == KD - 1),
                        skip_group_check=True,
                    )
                    if kd == KD - 1:
                        nc.vector.tensor_copy(
                            out=kt[:, ms : ms + ml], in_=kstate.pop(i)[:, :ml]
                        )

                def proj_fin():
                    nc.vector.tensor_copy(out=qt[:], in_=qps[:])
                    qts[h] = qt
                    kts[h] = kt

                q_mms = [(lambda kd: lambda: q_mm(kd))(kd) for kd in range(KD)]
                k_mms = [(lambda j: lambda: k_mm(j))(j) for j in range(2 * KD)]
                return q_mms, k_mms, proj_fin

            def emit_proj(h):
                # standalone proj for the first DEPTH heads (the rest are
                # interleaved into attend iterations)
                q_mms, k_mms, proj_fin = make_proj(h)
                for f in q_mms:
                    f()
                for f in k_mms:
                    f()
                proj_fin()

            def make_v_quarter(q):
                """Per-matmul emitters for the V projection of he-quarter q
                (heads 4q..4q+3): vall[m%P, mt, 512q:512(q+1)]. The psum tile
                allocates lazily per key-tile and casts on its last matmul,
                holding one ps2 buffer at a time."""
                state = {}

                def v_mm(j):
                    mt, kd = j // KD, j % KD
                    if kd == 0:
                        state[mt] = ps2.tile([P, 512], FP32, tag="ps512",
                                             name="vps")
                    nc.tensor.matmul(
                        state[mt][:],
                        encT[:, kd, mt * P : (mt + 1) * P],
                        wv[:, q, kd],
                        start=(kd == 0),
                        stop=(kd == KD - 1),
                        skip_group_check=True,
                    )
                    if kd == KD - 1:
                        nc.vector.tensor_copy(
                            out=vall[:, mt, q * 512 : (q + 1) * 512],
                            in_=state.pop(mt)[:],
                        )

                return [(lambda j: lambda: v_mm(j))(j) for j in range(MT * KD)]

            def emit_v_phase(quarters):
                for q in quarters:
                    for f in make_v_quarter(q):
                        f()

            def emit_attend(h, q_mms=(), k_mms=(), v_mms=(), fillers=()):
                # scores^T, softmax over free axis, headsT accum over key
                # tiles. The heads matmul is emitted DELAY iterations behind
                # the scores matmul (its stationary vsc comes off the softmax
                # chain ~2us later), and the next head's Q/K proj matmuls are
                # interleaved per-iteration so the PE's OOO exec window always
                # holds chain-independent work.
                DELAY = 5
                q_mms = list(q_mms)
                k_mms = list(k_mms)
                v_mms = list(v_mms)
                fillers = list(fillers)
                qt = qts.pop(h)
                kt = kts.pop(h)
                hps = psacc.tile([P, NDEV], FP32, tag="hacc", name="hps")
                abuf = {}
                vbuf = {}

                def emit_heads(mt):
                    nc.tensor.matmul(
                        hps[:],
                        vbuf[mt][:],
                        abuf.pop(mt)[:],
                        start=(mt == 0),
                        stop=(mt == MT - 1),
                        skip_group_check=True,
                    )
                    # tail output row: heads[512] column accumulates in a
                    # shared psum bank (read once after the last attend)
                    nc.tensor.matmul(
                        htail[:, h : h + 1],
                        vbuf.pop(mt)[:],
                        e512[:, mt, h : h + 1],
                        start=(mt == 0),
                        stop=(mt == MT - 1),
                        skip_group_check=True,
                    )

                for mt in range(MT):
                    tps = psnv.tile([P, NDEV], FP32, tag="psnv", name="tps")
                    nc.tensor.matmul(
                        tps[:],
                        kt[:, mt * P : (mt + 1) * P],
                        qt[:],
                        start=True,
                        stop=True,
                    )
                    a_sb = apool.tile([P, NDEV], BF16, tag="a", name="a_sb")
                    ssum = stats.tile([P, 1], FP32, tag="ssum", name="ssum")
                    nc.scalar.activation(
                        a_sb[:],
                        tps[:],
                        mybir.ActivationFunctionType.Exp,
                        scale=scale,
                        accum_out=ssum[:],
                    )
                    # denominators include the host tail-query column
                    ssumt = stats.tile([P, 1], FP32, tag="ssumt", name="ssumt")
                    nc.vector.tensor_tensor(
                        ssumt[:], ssum[:], e512[:, mt, h : h + 1],
                        mybir.AluOpType.add,
                    )
                    rcp = stats.tile([P, 1], FP32, tag="rcp", name="rcp")
                    nc.vector.reciprocal(rcp[:], ssumt[:])
                    vsc = apool.tile([P, E], BF16, tag="vsc", name="vsc")
                    nc.vector.tensor_scalar_mul(
                        vsc[:], vall[:, mt, h * E : (h + 1) * E], rcp[:]
                    )
                    abuf[mt] = a_sb
                    vbuf[mt] = vsc
                    if q_mms:
                        q_mms.pop(0)()
                    if mt >= DELAY:
                        emit_heads(mt - DELAY)
                    if k_mms:
                        k_mms.pop(0)()
                        k_mms.pop(0)()
                    if v_mms:
                        v_mms.pop(0)()
                        v_mms.pop(0)()
                    if fillers and mt % 2 == 1:
                        fillers.pop(0)()

                def finish():
                    # last DELAY heads matmuls + the multiT copy; the trailing
                    # chain latency hides under the next head's independent
                    # scores/proj matmuls via the PE OOO window.
                    for mt in range(MT - DELAY, MT):
                        emit_heads(mt)
                    nc.vector.tensor_copy(out=multiT[:, h, :], in_=hps[:])
                    for f in fillers:
                        f()

                return finish

            fin_parts = {}

            def emit_final_chunk(ns, nl, ds_, dl, half):
                # out[n, d] = concat_heads @ w_agg, split by head range: half 0
                # (heads < H1) stashes a bf16 partial, half 1 adds it on the
                # vector engine and streams the output tile.
                fps = ps2.tile([P, 512], FP32, tag="ps512", name="fps")
                hts = range(0, H1) if half == 0 else range(H1, H)
                for ht in hts:
                    nc.tensor.matmul(
                        fps[:nl, :dl],
                        multiT[:, ht, ns : ns + nl],
                        wagg[:, ht, ds_ : ds_ + dl],
                        start=(ht == hts[0]),
                        stop=(ht == hts[-1]),
                    )
                if half == 0:
                    part = fpool.tile(
                        [P, 512], BF16, tag=f"part{(ns // P) * 2 + ds_ // 512}",
                        name="part",
                    )
                    nc.vector.tensor_copy(out=part[:nl, :dl], in_=fps[:nl, :dl])
                    fin_parts[(ns, ds_)] = part
                else:
                    osb = opool.tile([P, 512], BF16, tag="osb", name="osb")
                    part = fin_parts.pop((ns, ds_))
                    nc.vector.tensor_tensor(
                        osb[:nl, :dl],
                        fps[:nl, :dl],
                        part[:nl, :dl],
                        mybir.AluOpType.add,
                    )
                    nc.sync.dma_start(out_d[ns : ns + nl, ds_ : ds_ + dl], osb[:nl, :dl])

            # Software pipeline: proj(h) runs DEPTH ahead of attend(h); the V
            # phase covers the encT/wv DMA stream. Each attend interleaves the
            # (h+DEPTH) head's proj matmuls per-iteration; trailing heads
            # matmuls are deferred into the next attend's start.
            for h in range(DEPTH):
                emit_proj(h)
            # V quarters 0-1 upfront (attends 0-7 read them); quarters 2-3
            # are deferred into attends 0-7 as an extra interleave stream so
            # the PE isn't gated on the tail of the wv DMA
            emit_v_phase([0, 1])
            vdef = make_v_quarter(2) + make_v_quarter(3)
            for h in range(DEPTH, H):
                q_mms, k_mms, proj_fin = make_proj(h)
                share, vdef = vdef[: 2 * MT], vdef[2 * MT :]
                fin_new = emit_attend(h - DEPTH, q_mms=q_mms, k_mms=k_mms,
                                      v_mms=share)
                proj_fin()
                fin_new()

            all_chunks = [
                (ns, nl, ds_, dl) for ns, nl in n_tiles for ds_, dl in d_chunks
            ]
            # Drain: the last DEPTH attends have no proj work left; interleave
            # final chunks over heads 0..H1-1 as PE filler.
            drain = list(range(H - DEPTH, H))
            per = (len(all_chunks) + len(drain) - 1) // len(drain)
            for i, h in enumerate(drain):
                cs = all_chunks[i * per : (i + 1) * per]
                fils = [(lambda c: lambda: emit_final_chunk(*c, 0))(c) for c in cs]
                fin = emit_attend(h, fillers=fils[:-1])
                fils[-1]()
                fin()
            # ship the tail heads column while the last final chunks run
            tailc = opool.tile([P, H], BF16, tag="tailc", name="tailc")
            nc.vector.tensor_copy(out=tailc[:], in_=htail[:])
            nc.gpsimd.dma_start(tail_d[:], tailc[:])
            for c in all_chunks:
                emit_final_chunk(*c, 1)

    nc.compile()
    return nc


def kernel(x, encoder_context, attention_mask, wq, wk, wv, w_agg, current_index):
    global LAST_RESULTS
    x = np.asarray(x)
    enc = np.asarray(encoder_context)
    wq = np.asarray(wq)
    wk = np.asarray(wk)
    wv = np.asarray(wv)
    w_agg = np.asarray(w_agg)
    ci = int(np.asarray(current_index))
    NV = min(ci + 1, N - 1)
    NDEV = NV - 1
    assert NV % P == 1 and NV > P, "kernel tuned for NV = k*128 + 1 (spec: 513)"

    nc = _cache.get(NV)
    if nc is None:
        nc = _build(NV)
        _cache[NV] = nc

    bf = ml_dtypes.bfloat16
    # weight layouts: see dram tensor declarations in _build
    wq_h = np.ascontiguousarray(wq.reshape(H, KD, P, E).transpose(2, 0, 1, 3)).astype(bf)
    wk_h = np.ascontiguousarray(wk.reshape(H, KD, P, E).transpose(2, 0, 1, 3)).astype(bf)
    wv_h = np.ascontiguousarray(
        wv.reshape(4, H // 4, KD, P, E).transpose(3, 0, 2, 1, 4)
    ).astype(bf)
    wagg_h = np.ascontiguousarray(w_agg.reshape(H, P, D).transpose(1, 0, 2)).astype(bf)

    scale = 1.0 / np.sqrt(np.float32(E))
    in_maps = []
    for b in range(B):
        xT_b = np.ascontiguousarray(
            x[b, :NDEV, :].T.reshape(KD, P, NDEV).transpose(1, 0, 2)
        ).astype(bf)
        encT_b = np.ascontiguousarray(
            enc[b].T.reshape(KD, P, N).transpose(1, 0, 2)
        ).astype(bf)
        # Tail-query score row, computed exactly on the host:
        #   q512[h] = x[512] @ wq[h];  s512[h, m] = enc[m] . (wk[h] @ q512[h])
        q512 = np.einsum("d,hde->he", x[b, NDEV], wq, optimize=True)
        t = np.einsum("hde,he->hd", wk, q512, optimize=True)
        s512 = enc[b].astype(np.float32) @ t.T.astype(np.float32)  # [M, H]
        e512_b = np.ascontiguousarray(
            np.exp(s512 * scale).reshape(MT, P, H).transpose(1, 0, 2)
        ).astype(bf)
        in_maps.append(
            {
                "xT": xT_b,
                "encT": encT_b,
                "wq": wq_h,
                "wk": wk_h,
                "wv": wv_h,
                "wagg": wagg_h,
                "e512": e512_b,
            }
        )

    if TRACE:
        _ensure_ntff_hook()
    res = run_bass_kernel_spmd(
        nc, in_maps, core_ids=list(range(NCORES)), trace=TRACE
    )
    LAST_RESULTS = res

    out = np.zeros((B, N, D), np.float32)
    wagg_f = w_agg.astype(np.float32)
    for b in range(B):
        r = res.results[b]
        out[b, :NDEV, :] = np.asarray(r["out"]).astype(np.float32)
        # tail_he[p, h] = heads[512, h*E + p]
        t = np.asarray(r["tail_he"]).astype(np.float32)
        out[b, NDEV, :] = t.T.reshape(H * E) @ wagg_f
    return out
